# revision 23
# baseline (speedup 1.0000x reference)
"""Trainium2 Bass kernel for NemotronFlash Mamba decoder layer.

Sharding: 8 cores = 2 batches x 4 sequence shards of 512 tokens.
All compute is shard-local except the SSD inter-chunk state, which is
exchanged via one AllGather of (L_k, D_k) within each 4-core batch group.

v2 restructure vs baseline:
- in-proj computes xBC tiles first, dt, then z; conv/dt/states/collective
  start early and overlap the z matmuls.
- activations batched by function (fewer ACT table loads); Rsqrt/Softplus.
- Y produced directly in [E, tokens] layout via PSUM accumulation of
  Y_diag+Y_off per head (no yT DMA transposes, no separate add pass).
- exp(acs) broadcast via a single tiny exp + bf16 DRAM broadcast load.
- gpsimd (Pool engine) offloads part of conv and elementwise work.
"""
import sys
import numpy as np

sys.path.insert(0, "/opt/trn_rl_repo")

from contextlib import ExitStack  # noqa: E402
import ml_dtypes  # noqa: E402
import concourse.bass as bass  # noqa: E402
import concourse.mybir as mybir  # noqa: E402
import concourse.tile as tile  # noqa: E402
from concourse import bacc  # noqa: E402
from concourse.bass_utils import run_bass_kernel_spmd  # noqa: E402

F32 = mybir.dt.float32
BF16 = mybir.dt.bfloat16
AF = mybir.ActivationFunctionType
OP = mybir.AluOpType

H = 1024
E = 2048
NH = 32
P = 64
NST = 128          # d_state
KC = 4             # d_conv
Q = 128            # chunk len
FF = 4096
CONV = E + 2 * NST          # 2304
D_IN = 2 * E + 2 * NST + NH  # 4384
EPS = 1e-6
NEPS = 1e-5
LSEQ = 512         # tokens per shard
NCHUNK = LSEQ // Q  # 4
NROW = 5           # 5 row tiles of 128 = 640 padded rows
HALO = 3
NCORES = 8
NEG = -1.0e30

NZT = E // Q       # 16 z tiles
NXT = CONV // Q    # 18 xBC tiles
NMT = NXT + 1 + NZT  # 35 in-proj tiles (18 xBC + 1 dt + 16 z)
NKH = H // Q       # 8 k tiles over H
NKE = E // Q       # 16 k tiles over E
NFT = FF // Q      # 32 FF tiles
HG = 8             # heads per group
NG = NH // HG      # 4 groups

# in-proj tile order: xBC tiles 0..17, dt (32 rows), z tiles 0..15
TSIZES = [128] * NXT + [32] + [128] * NZT


def make_groups():
    groups = []
    m = 0
    while m < NMT:
        g0 = m
        cols = 0
        while m < NMT and cols + TSIZES[m] <= 512:
            cols += TSIZES[m]
            m += 1
        groups.append((g0, m, cols))
    return groups


GROUPS = make_groups()
NGRP = len(GROUPS)  # 9


def row_bcast(ap_row, parts=128):
    """AP broadcasting a [1, n] row across `parts` partitions (step-0)."""
    return bass.AP(tensor=ap_row.tensor, offset=ap_row.offset,
                   ap=[[0, parts]] + [list(x) for x in ap_row.ap[1:]])


def colbc(src_ap, n, rep):
    # [128, n, rep] broadcast of per-head columns along a new axis
    return bass.AP(tensor=src_ap.tensor, offset=src_ap.offset,
                   ap=[list(src_ap.ap[0])] + [[1, n], [0, rep]])


def rowbc(src_ap, rep, n):
    # [128, rep, n] broadcast of a [128, n] tile along middle axis
    return bass.AP(tensor=src_ap.tensor, offset=src_ap.offset,
                   ap=[list(src_ap.ap[0])] + [[0, rep], [1, n]])


def build_program(dvals):
    nc = bacc.Bacc("TRN2", target_bir_lowering=False, debug=False,
                   num_devices=NCORES)

    hs_in = nc.dram_tensor("hs", [NROW * 128, H], F32, kind="ExternalInput")
    wiT = nc.dram_tensor("wiT", [NGRP * 128, NKH * 512], BF16,
                         kind="ExternalInput")
    woT = nc.dram_tensor("woT", [E, H], BF16, kind="ExternalInput")
    wgT = nc.dram_tensor("wgT", [NFT * 128, NKH * 128], BF16,
                         kind="ExternalInput")
    wuT = nc.dram_tensor("wuT", [NFT * 128, NKH * 128], BF16,
                         kind="ExternalInput")
    wdT = nc.dram_tensor("wdT", [FF, H], BF16, kind="ExternalInput")
    wconv = nc.dram_tensor("wconv", [128, NXT * KC], F32, kind="ExternalInput")
    bconv = nc.dram_tensor("bconv", [128, NXT], F32, kind="ExternalInput")
    avec = nc.dram_tensor("avec", [NH, 1], F32, kind="ExternalInput")
    dtb = nc.dram_tensor("dtb", [NH, 1], F32, kind="ExternalInput")
    mask8 = nc.dram_tensor("mask8", [128, 8], F32, kind="ExternalInput")
    negu = nc.dram_tensor("negu", [128, 128], F32, kind="ExternalInput")
    idf32 = nc.dram_tensor("idf32", [128, 128], F32, kind="ExternalInput")
    dexpc_in = nc.dram_tensor("dexpc", [128, NZT], F32, kind="ExternalInput")
    out_d = nc.dram_tensor("out", [LSEQ, H], F32, kind="ExternalOutput")

    with tile.TileContext(nc) as tc, ExitStack() as stack:
        consts = stack.enter_context(tc.tile_pool(name="consts", bufs=1))
        wconv_sb = consts.tile([128, NXT * KC], F32)
        nc.sync.dma_start(out=wconv_sb[:], in_=wconv[:])
        bconv_sb = consts.tile([128, NXT], F32)
        nc.sync.dma_start(out=bconv_sb[:], in_=bconv[:])
        avec_sb = consts.tile([NH, 1], F32)
        nc.sync.dma_start(out=avec_sb[:], in_=avec[:])
        dtb_sb = consts.tile([NH, 1], F32)
        nc.sync.dma_start(out=dtb_sb[:], in_=dtb[:])
        mask_sb = consts.tile([128, 8], F32)
        nc.sync.dma_start(out=mask_sb[:], in_=mask8[:])
        negu_sb = consts.tile([128, 128], F32)
        nc.sync.dma_start(out=negu_sb[:], in_=negu[:])
        negu_bf = consts.tile([128, 128], BF16)
        nc.vector.tensor_copy(negu_bf[:], negu_sb[:])
        id_sb = consts.tile([128, 128], F32)
        nc.sync.dma_start(out=id_sb[:], in_=idf32[:])
        dexpc_sb = consts.tile([128, NZT], F32)
        nc.sync.dma_start(out=dexpc_sb[:], in_=dexpc_in[:])
        ones_bf = consts.tile([128, 1], BF16)
        nc.vector.memset(ones_bf[:], 1.0)
        zero32 = consts.tile([NH, Q], F32)
        nc.vector.memset(zero32[:], 0.0)
        epsc = consts.tile([128, 1], F32)
        nc.vector.memset(epsc[:], EPS)
        nepsc = consts.tile([128, 1], F32)
        nc.vector.memset(nepsc[:], NEPS)

        ccdram = stack.enter_context(
            tc.tile_pool(name="ccdram", bufs=1, space="DRAM"))
        cc_in = ccdram.tile([128, E + 1], BF16)
        cc_out = ccdram.tile([4, 128, E + 1], BF16)
        acsR_d = ccdram.tile([NCHUNK * NH, Q], F32)
        eacsR_d = ccdram.tile([NCHUNK * NH, Q], BF16)
        rs_d = ccdram.tile([1, LSEQ], F32)
        acst_d = ccdram.tile([1, NCHUNK * 3 * NH], F32)
        drow_d = ccdram.tile([1, 4 * NH], F32)

        big = stack.enter_context(tc.tile_pool(name="big", bufs=1))
        h2 = big.tile([128, NCHUNK, H], F32)
        h2nT = big.tile([128, NKH, LSEQ], BF16)

        es_P1 = ExitStack()                        # A .. end of G
        pP1 = es_P1.enter_context(tc.tile_pool(name="pP1", bufs=1))
        hT = pP1.tile([128, NKH, NROW * 128], BF16)      # h^T  [H, 640]
        szT = pP1.tile([128, NZT, LSEQ], BF16)           # silu(z)^T
        dtraw = pP1.tile([NH, LSEQ], F32)

        # ---------------- Phase A: rmsnorm1 + h^T (batched) ----------------
        with tc.tile_pool(name="pA", bufs=5) as pA, \
             tc.tile_pool(name="stat", bufs=5) as stat:
            hsts, rss = [], []
            for r in range(NROW):
                hst = pA.tile([128, H], F32, tag="hst", name=f"hst{r}")
                nc.sync.dma_start(out=hst[:],
                                  in_=hs_in[r * 128:(r + 1) * 128, :])
                hsts.append(hst)
            ssums = []
            for r in range(NROW):
                sq = pA.tile([128, H], F32, tag="sq", bufs=2, name=f"sq{r}")
                ssum = stat.tile([128, 1], F32, tag="ssum", name=f"ssum{r}")
                nc.scalar.activation(out=sq[:], in_=hsts[r][:],
                                     func=AF.Square, accum_out=ssum[:])
                ssums.append(ssum)
            for r in range(NROW):
                rs = stat.tile([128, 1], F32, tag="rs", name=f"rs{r}")
                nc.scalar.activation(out=rs[:], in_=ssums[r][:],
                                     func=AF.Ln, scale=1.0 / H,
                                     bias=epsc[:])
                rss.append(rs)
            for r in range(NROW):
                nc.scalar.activation(out=rss[r][:], in_=rss[r][:],
                                     func=AF.Exp, scale=-0.5)
            for r in range(NROW):
                hbf = pA.tile([128, H], BF16, tag="hbf", name=f"hbf{r}",
                              bufs=3)
                nc.vector.tensor_scalar_mul(hbf[:], hsts[r][:], rss[r][:])
                nc.sync.dma_start_transpose(
                    hT[:, :, r * 128:(r + 1) * 128], hbf[:])

        es_yt = ExitStack()                        # B .. end of G
        pYT = es_yt.enter_context(tc.tile_pool(name="pYT", bufs=1))
        yT = pYT.tile([128, NKE, LSEQ], BF16)

        es_cf = ExitStack()                        # B .. F
        pCF = es_cf.enter_context(tc.tile_pool(name="pCF", bufs=1))
        dtacsT = pCF.tile([128, NCHUNK, 3 * NH], F32)
        cstates = pCF.tile([128, NCHUNK, E], BF16)
        alast = pCF.tile([128, NCHUNK, NH], F32)
        wdtb = pCF.tile([128, NCHUNK, NH], BF16)
        dcstb = pCF.tile([128, NCHUNK, NH], BF16)
        dt_sb = pCF.tile([NH, LSEQ], F32)
        acs = pCF.tile([NH, LSEQ], F32)

        es_df = ExitStack()                        # B .. F
        pDF = es_df.enter_context(tc.tile_pool(name="pDF", bufs=1))
        xcbc = pDF.tile([128, 2, LSEQ], BF16)
        G_sb = pDF.tile([128, NCHUNK, Q], BF16)
        x_tm = pDF.tile([128, NCHUNK, E], BF16)
        B_tm = pDF.tile([128, NCHUNK, NST], BF16)

        es_dx = ExitStack()                        # B .. yT prefill (xc)
        pDX = es_dx.enter_context(tc.tile_pool(name="pDX", bufs=1))
        xc = pDX.tile([128, NZT, LSEQ], BF16)

        # ---------------- Phase B + C + D + E interleaved ----------------
        es_wip = ExitStack()
        wip = es_wip.enter_context(tc.tile_pool(name="wip", bufs=2))
        psB = ExitStack()
        psBp = psB.enter_context(tc.tile_pool(name="psB", bufs=4,
                                              space="PSUM"))
        psBh = psB.enter_context(tc.tile_pool(name="psBh", bufs=2,
                                              space="PSUM"))
        es_xbc = ExitStack()
        xbcf = es_xbc.enter_context(tc.tile_pool(name="xbcf", bufs=1))
        xbc = xbcf.tile([128, NXT, HALO + LSEQ], BF16)

        def conv_tile(j):
            eng = nc.vector
            acc = None
            for k in range(KC):
                if k == 0:
                    acc = xbcf.tile([128, LSEQ], F32, tag="cacc", bufs=3,
                                    name=f"cacc{j}_0")
                    eng.tensor_scalar_mul(
                        acc[:], xbc[:, j, 0:LSEQ],
                        wconv_sb[:, j * KC:j * KC + 1])
                else:
                    acc2 = xbcf.tile([128, LSEQ], F32, tag="cacc", bufs=3,
                                     name=f"cacc{j}_{k}")
                    eng.scalar_tensor_tensor(
                        out=acc2[:], in0=xbc[:, j, k:k + LSEQ],
                        scalar=wconv_sb[:, j * KC + k:j * KC + k + 1],
                        in1=acc[:], op0=OP.mult, op1=OP.add)
                    acc = acc2
            xdst = (xc[:, j, :] if j < NZT else xcbc[:, j - NZT, :])
            nc.scalar.activation(out=xdst, in_=acc[:], func=AF.Silu,
                                 bias=bconv_sb[:, j:j + 1])
            if j < NZT:
                nc.sync.dma_start_transpose(
                    x_tm[:, :, j * 128:(j + 1) * 128], xc[:, j, :])
            elif j == NZT:
                nc.sync.dma_start_transpose(B_tm[:], xcbc[:, 0, :])

        def emit_group(gi):
            g0, g1, cols = GROUPS[gi]
            wi_g = wip.tile([128, NKH, 512], BF16, tag="wi", name=f"wi{gi}")
            nc.sync.dma_start(out=wi_g[:],
                              in_=wiT[gi * 128:(gi + 1) * 128, :])
            moff = 0
            for mm in range(g0, g1):
                mrows = TSIZES[mm]
                ps = psBp.tile([128, LSEQ], F32, tag="ps")
                for k in range(NKH):
                    nc.tensor.matmul(
                        ps[:mrows, :],
                        wi_g[:, k, moff:moff + mrows],
                        hT[:, k, HALO:HALO + LSEQ],
                        start=(k == 0), stop=(k == NKH - 1))
                if mm < NXT:                      # xBC tile
                    j = mm
                    nc.scalar.copy(xbc[:, j, HALO:], ps[:])
                    psh = psBh.tile([128, HALO], F32, tag="psh")
                    for k in range(NKH):
                        nc.tensor.matmul(
                            psh[:], wi_g[:, k, moff:moff + 128],
                            hT[:, k, 0:HALO],
                            start=(k == 0), stop=(k == NKH - 1))
                    nc.scalar.copy(xbc[:, j, 0:HALO], psh[:])
                    conv_tile(j)
                elif mm == NXT:                   # dt tile
                    nc.vector.tensor_copy(dtraw[:], ps[:NH, :])
                else:                             # z tile: silu from PSUM
                    mz = mm - NXT - 1
                    nc.scalar.activation(out=szT[:, mz, :], in_=ps[:],
                                         func=AF.Silu)
                moff += mrows

        # Part 1: xBC + dt + z0 (groups 0..4), conv pipelined per tile
        for gi in range(5):
            emit_group(gi)
        es_xbc.close()

        # ---------------- Phase C: dt path ----------------
        with tc.tile_pool(name="pC", bufs=2) as pC:
            e1 = pC.tile([NH, LSEQ], F32, tag="e1")
            nc.scalar.activation(out=e1[:], in_=dtraw[:], func=AF.Exp,
                                 bias=dtb_sb[:])
            nc.vector.tensor_scalar_add(e1[:], e1[:], 1.0)
            nc.scalar.activation(out=dt_sb[:], in_=e1[:], func=AF.Ln)
            lndt = pC.tile([NH, LSEQ], F32, tag="lndt")
            nc.scalar.activation(out=lndt[:], in_=dt_sb[:], func=AF.Ln)
            dA = pC.tile([NH, LSEQ], F32, tag="dA")
            nc.vector.tensor_scalar_mul(dA[:], dt_sb[:], avec_sb[:])
            for c in range(NCHUNK):
                nc.vector.tensor_tensor_scan(
                    acs[:, c * Q:(c + 1) * Q], dA[:, c * Q:(c + 1) * Q],
                    zero32[:], 0.0, OP.add, OP.add)
            av = acsR_d[:]
            nc.sync.dma_start(
                out=bass.AP(tensor=av.tensor, offset=av.offset,
                            ap=[[Q, NH], [NH * Q, NCHUNK], [1, Q]]),
                in_=acs[:].rearrange("h (c q) -> h c q", c=NCHUNK))
            stk = pC.tile([3 * NH, LSEQ], F32, tag="stk")
            nc.vector.tensor_copy(stk[0:NH, :], dt_sb[:])
            nc.vector.tensor_copy(stk[NH:2 * NH, :], acs[:])
            nc.vector.tensor_copy(stk[2 * NH:3 * NH, :], lndt[:])
            with tc.tile_pool(name="psC", bufs=2, space="PSUM") as psC:
                for c in range(NCHUNK):
                    pst = psC.tile([128, 3 * NH], F32, tag="pst")
                    nc.tensor.transpose(pst[:], stk[:, c * Q:(c + 1) * Q],
                                        id_sb[0:3 * NH, 0:3 * NH])
                    nc.scalar.copy(dtacsT[:, c, :], pst[:])
            nc.sync.dma_start(out=acst_d[:], in_=dtacsT[127:128, :, :])
            at_ = acst_d[:]
            nc.sync.dma_start(
                out=alast[:],
                in_=bass.AP(tensor=at_.tensor, offset=at_.offset + NH,
                            ap=[[0, 128], [3 * NH, NCHUNK], [1, NH]]))
            dec0 = pC.tile([128, NCHUNK, NH], F32, tag="dec0")
            nc.vector.scalar_tensor_tensor(
                out=dec0[:], in0=dtacsT[:, :, NH:2 * NH], scalar=-1.0,
                in1=alast[:], op0=OP.mult, op1=OP.add)
            # batched Exp set: decT, dcstb, eacs, dkcol
            decT = pC.tile([128, NCHUNK, NH], F32, tag="decT")
            nc.scalar.activation(out=decT[:], in_=dec0[:], func=AF.Exp)
            nc.scalar.activation(out=dcstb[:], in_=alast[:], func=AF.Exp)
            eacs = pC.tile([NH, LSEQ], BF16, tag="eacs")
            nc.scalar.activation(out=eacs[:], in_=acs[:], func=AF.Exp)
            acs4 = acs[:].rearrange("p (c q) -> p c q", c=NCHUNK)[:, :, Q - 1]
            asum = pC.tile([NH, 1], F32, tag="asum")
            nc.vector.tensor_reduce(asum[:], acs4, axis=mybir.AxisListType.X,
                                    op=OP.add)
            dkcol = pC.tile([NH, 1], BF16, tag="dkcol")
            nc.scalar.activation(out=dkcol[:], in_=asum[:], func=AF.Exp)
            eav = eacsR_d[:]
            nc.sync.dma_start(
                out=bass.AP(tensor=eav.tensor, offset=eav.offset,
                            ap=[[Q, NH], [NH * Q, NCHUNK], [1, Q]]),
                in_=eacs[:].rearrange("h (c q) -> h c q", c=NCHUNK))
            nc.vector.tensor_mul(wdtb[:], decT[:], dtacsT[:, :, 0:NH])
            dcol = pCF.tile([128, NCHUNK, NH], F32, name="dcol")
            nc.vector.tensor_sub(dcol[:], dtacsT[:, :, NH:2 * NH],
                                 dtacsT[:, :, 2 * NH:3 * NH])

            # G_sb = B^T C per chunk (unmasked; eL handles causality)
            with tc.tile_pool(name="psGm", bufs=2, space="PSUM") as psGm:
                for c in range(NCHUNK):
                    gps = psGm.tile([128, Q], F32, tag="gps")
                    nc.tensor.matmul(gps[:], xcbc[:, 0, c * Q:(c + 1) * Q],
                                     xcbc[:, 1, c * Q:(c + 1) * Q],
                                     start=True, stop=True)
                    nc.vector.tensor_copy(G_sb[:, c, :], gps[:])

            emit_group(5)

            # ------------- Phase E: states + collective -------------
            with tc.tile_pool(name="psE", bufs=2, space="PSUM") as psE, \
                 tc.tile_pool(name="pE", bufs=2) as pE:
                xv = [x_tm[:, c, :].rearrange("p (h q) -> p h q", h=NH)
                      for c in range(NCHUNK)]
                for c in range(NCHUNK):
                    xdd = pE.tile([128, NH, P], BF16, tag="xdd",
                                  name=f"xdd{c}", bufs=1)
                    eng = nc.gpsimd if c == 3 else nc.vector
                    eng.tensor_mul(xdd[:], xv[c],
                                   colbc(wdtb[:, c, :], NH, P))
                    for g in range(NG):
                        ps_st = psE.tile([128, 512], F32, tag="ps_st")
                        nc.tensor.matmul(
                            ps_st[:], B_tm[:, c, :],
                            xdd[:, g * HG:(g + 1) * HG, :],
                            start=True, stop=True)
                        if g % 2 == 0:
                            nc.scalar.copy(
                                cstates[:, c, g * 512:(g + 1) * 512],
                                ps_st[:])
                        else:
                            nc.vector.tensor_copy(
                                cstates[:, c, g * 512:(g + 1) * 512],
                                ps_st[:])
                # L combine via suffix decay products
                dsuf = pE.tile([128, NCHUNK, NH], F32, tag="dsuf", bufs=1)
                nc.vector.tensor_copy(dsuf[:, 3, :], dcstb[:, 3, :])
                nc.vector.tensor_mul(dsuf[:, 2, :], dsuf[:, 3, :],
                                     dcstb[:, 2, :])
                nc.vector.tensor_mul(dsuf[:, 1, :], dsuf[:, 2, :],
                                     dcstb[:, 1, :])
                cs_v = [cstates[:, c, :].rearrange("p (h q) -> p h q", h=NH)
                        for c in range(NCHUNK)]
                m0 = pE.tile([128, NH, P], BF16, tag="lwork", bufs=3,
                             name="m0")
                nc.vector.tensor_mul(m0[:], cs_v[0],
                                     colbc(dsuf[:, 1, :], NH, P))
                m1 = pE.tile([128, NH, P], BF16, tag="lwork", bufs=3,
                             name="m1")
                nc.gpsimd.tensor_mul(m1[:], cs_v[1],
                                     colbc(dsuf[:, 2, :], NH, P))
                m2 = pE.tile([128, NH, P], BF16, tag="lwork", bufs=3,
                             name="m2")
                nc.vector.tensor_mul(m2[:], cs_v[2],
                                     colbc(dsuf[:, 3, :], NH, P))
                nc.vector.tensor_add(m0[:], m0[:], m1[:])
                nc.gpsimd.tensor_add(m2[:], m2[:], cs_v[3])
                Lbf = pE.tile([128, E], BF16, tag="lbf", bufs=1)
                nc.vector.tensor_add(
                    Lbf[:].rearrange("p (h q) -> p h q", h=NH),
                    m0[:], m2[:])
                nc.gpsimd.dma_start(out=cc_in[:, 0:E], in_=Lbf[:])
                nc.gpsimd.dma_start(out=cc_in[0:NH, E:E + 1], in_=dkcol[:])
                nc.gpsimd.collective_compute(
                    "AllGather", OP.bypass,
                    replica_groups=[[0, 1, 2, 3], [4, 5, 6, 7]],
                    ins=[cc_in.opt()], outs=[cc_out.opt()])

        # Part 2: remaining z tiles (groups 6..8)
        for gi in range(6, NGRP):
            emit_group(gi)
        psB.close()
        es_wip.close()

        # yT prefill with D-skip term: yT[:, j, :] = D * x
        for j in range(NZT):
            nc.vector.tensor_scalar_mul(yT[:, j, :], xc[:, j, :],
                                        dexpc_sb[:, j:j + 1])
        es_dx.close()

        es_mt = ExitStack()                        # Fprep .. F
        pMT = es_mt.enter_context(tc.tile_pool(name="pMT", bufs=1))
        mt_sb = pMT.tile([128, NCHUNK, NH, Q], BF16)

        # ------- Phase F-prep: mt (overlaps collective; vector/scalar only,
        # gpsimd queue is blocked inside collective_compute) -------
        with tc.tile_pool(name="pFp", bufs=2) as pFp:
            for c in range(NCHUNK):
                for g in range(NG):
                    h0 = g * HG
                    R_all = pFp.tile([128, HG, Q], F32, tag="rall")
                    av2 = acsR_d[:]
                    nc.sync.dma_start(
                        out=R_all[:],
                        in_=bass.AP(tensor=av2.tensor,
                                    offset=av2.offset + (c * NH + h0) * Q,
                                    ap=[[0, 128], [1, HG * Q]]))
                    # seg = acs_q - acs_s  (f32 in, bf16 out)
                    # seg = acs_q - acs_s + ln(dt_s)  (dt folded in exp)
                    seg = pFp.tile([128, HG, Q], BF16, tag="seg")
                    nc.vector.scalar_tensor_tensor(
                        out=seg[:],
                        in0=colbc(dcol[:, c, h0:h0 + HG], HG, Q),
                        scalar=-1.0, in1=R_all[:],
                        op0=OP.mult, op1=OP.add)
                    segm = pFp.tile([128, HG, Q], BF16, tag="segm")
                    nc.vector.tensor_add(segm[:], seg[:],
                                         rowbc(negu_bf[:], HG, Q))
                    eL = pFp.tile([128, HG, Q], BF16, tag="eL")
                    nc.scalar.activation(out=eL[:], in_=segm[:], func=AF.Exp)
                    nc.vector.tensor_mul(
                        mt_sb[:, c, h0:h0 + HG, :], eL[:],
                        rowbc(G_sb[:, c, :], HG, Q))

        # ---------------- S_init combine ----------------
        es_sb = ExitStack()
        sbfp = es_sb.enter_context(tc.tile_pool(name="sbfp", bufs=2))
        Sbf = None
        with tc.tile_pool(name="pS", bufs=1) as pS:
            Lg = pS.tile([128, 4, E], BF16, tag="Lg")
            Dg = pS.tile([NH, 4], BF16, tag="Dg")
            for j in range(4):
                nc.sync.dma_start(out=Lg[:, j, :], in_=cc_out[j, :, 0:E])
                nc.sync.dma_start(out=Dg[:, j:j + 1],
                                  in_=cc_out[j, 0:NH, E:E + 1])
            deff = pS.tile([NH, 4], F32, tag="deff")
            for j in range(4):
                nc.vector.scalar_tensor_tensor(
                    out=deff[:, j:j + 1], in0=Dg[:, j:j + 1],
                    scalar=mask_sb[0:NH, j:j + 1],
                    in1=mask_sb[0:NH, 4 + j:5 + j],
                    op0=OP.mult, op1=OP.add)
            # coef[j] = mask_j * prod_{i>j} deff_i (suffix products, tiny)
            suf = pS.tile([NH, 4], F32, tag="suf")
            nc.vector.memset(suf[:, 3:4], 1.0)
            nc.vector.tensor_copy(suf[:, 2:3], deff[:, 3:4])
            nc.vector.tensor_mul(suf[:, 1:2], deff[:, 2:3], suf[:, 2:3])
            nc.vector.tensor_mul(suf[:, 0:1], deff[:, 1:2], suf[:, 1:2])
            coef = pS.tile([NH, 4], F32, tag="coef")
            nc.vector.tensor_mul(coef[:], suf[:], mask_sb[0:NH, 0:4])
            for j in range(4):
                nc.sync.dma_start(out=drow_d[0:1, j * NH:(j + 1) * NH],
                                  in_=coef[:, j:j + 1])
            dbc = pS.tile([128, 4 * NH], F32, tag="dbc")
            nc.sync.dma_start(out=dbc[:], in_=row_bcast(drow_d[0:1, :]))
            lgv = [Lg[:, j, :].rearrange("p (h q) -> p h q", h=NH)
                   for j in range(4)]
            ps0 = pS.tile([128, NH, P], BF16, tag="sw0")
            nc.vector.tensor_mul(ps0[:], lgv[0], colbc(dbc[:, 0:NH], NH, P))
            ps1 = pS.tile([128, NH, P], BF16, tag="sw1")
            nc.gpsimd.tensor_mul(ps1[:], lgv[1],
                                 colbc(dbc[:, NH:2 * NH], NH, P))
            ps2 = pS.tile([128, NH, P], BF16, tag="sw2")
            nc.vector.tensor_mul(ps2[:], lgv[2],
                                 colbc(dbc[:, 2 * NH:3 * NH], NH, P))
            ps3 = pS.tile([128, NH, P], BF16, tag="sw3")
            nc.gpsimd.tensor_mul(ps3[:], lgv[3],
                                 colbc(dbc[:, 3 * NH:4 * NH], NH, P))
            a01 = pS.tile([128, NH, P], BF16, tag="sa01")
            nc.vector.tensor_add(a01[:], ps0[:], ps1[:])
            a23 = pS.tile([128, NH, P], BF16, tag="sa23")
            nc.gpsimd.tensor_add(a23[:], ps2[:], ps3[:])
            Sbf = sbfp.tile([128, E], BF16, tag="sbf", name="sbf0")
            nc.vector.tensor_add(
                Sbf[:].rearrange("p (h q) -> p h q", h=NH), a01[:], a23[:])

        # ---------------- Phase F: merged Y_diag+Y_off -> yT ----------
        with tc.tile_pool(name="pF2", bufs=2) as pF2, \
             tc.tile_pool(name="psY", bufs=8, space="PSUM") as psY:
            for c in range(NCHUNK):
                # Ct = exp(acs) * C  (independent of collective)
                Ct = pF2.tile([128, NH, Q], BF16, tag="ct", bufs=1)
                for g in range(NG):
                    h0 = g * HG
                    eA = pF2.tile([128, HG, Q], BF16, tag="ear")
                    eav2 = eacsR_d[:]
                    nc.sync.dma_start(
                        out=eA[:],
                        in_=bass.AP(tensor=eav2.tensor,
                                    offset=eav2.offset + (c * NH + h0) * Q,
                                    ap=[[0, 128], [1, HG * Q]]))
                    eng = nc.gpsimd if g == 3 else nc.vector
                    eng.tensor_mul(
                        Ct[:, h0:h0 + HG, :], eA[:],
                        rowbc(xcbc[:, 1, c * Q:(c + 1) * Q], HG, Q))
                for t in range(4):
                    yps = psY.tile([128, 512], F32, tag="yps")
                    for jj in range(4):
                        j = 4 * t + jj
                        for hh in range(2):
                            h = 2 * j + hh
                            reg = yps[hh * 64:(hh + 1) * 64,
                                      jj * 128:(jj + 1) * 128]
                            nc.tensor.matmul(
                                reg, x_tm[:, c, h * P:(h + 1) * P],
                                mt_sb[:, c, h, :],
                                start=True, stop=False)
                            nc.tensor.matmul(
                                reg, Sbf[:, h * P:(h + 1) * P],
                                Ct[:, h, :],
                                start=False, stop=True)
                    for jj in range(4):
                        j = 4 * t + jj
                        nc.vector.tensor_add(
                            yT[:, j, c * Q:(c + 1) * Q],
                            yT[:, j, c * Q:(c + 1) * Q],
                            yps[:, jj * 128:(jj + 1) * 128])
                # next state (chain) - after this chunk's Y_off matmuls
                if c < NCHUNK - 1:
                    Snext = sbfp.tile([128, E], BF16, tag="sbf",
                                      name=f"sbf{c + 1}")
                    st = pF2.tile([128, NH, P], BF16, tag="stmp", bufs=1)
                    nc.vector.tensor_mul(
                        st[:], Sbf[:].rearrange("p (h q) -> p h q", h=NH),
                        colbc(dcstb[:, c, :], NH, P))
                    nc.vector.tensor_add(
                        Snext[:].rearrange("p (h q) -> p h q", h=NH), st[:],
                        cstates[:, c, :].rearrange("p (h q) -> p h q", h=NH))
                    Sbf = Snext
        es_sb.close()
        es_mt.close()
        es_df.close()
        es_cf.close()

        # ---------------- Phase G: gating + norm + out-proj ----------------
        with tc.tile_pool(name="pGa", bufs=2) as pGa, \
             tc.tile_pool(name="ygP", bufs=1) as ygP, \
             tc.tile_pool(name="woP", bufs=1) as woP, \
             tc.tile_pool(name="psN", bufs=1, space="PSUM") as psN, \
             tc.tile_pool(name="psO", bufs=3, space="PSUM") as psO, \
             tc.tile_pool(name="stat2", bufs=4) as stat2:
            wo_sb = woP.tile([128, NKE, H], BF16)
            for k in range(NKE):
                nc.sync.dma_start(out=wo_sb[:, k, :],
                                  in_=woT[k * 128:(k + 1) * 128, :])
            yg = ygP.tile([128, NKE, LSEQ], BF16)
            for mz in range(NKE):
                eng = nc.gpsimd if mz % 4 == 3 else nc.vector
                eng.tensor_mul(yg[:, mz, :], yT[:, mz, :], szT[:, mz, :])
            sqps = psN.tile([128, LSEQ], F32)
            for mz in range(NKE):
                g2 = pGa.tile([128, LSEQ], BF16, tag="g2", bufs=3)
                nc.scalar.activation(out=g2[:], in_=yg[:, mz, :],
                                     func=AF.Square)
                nc.tensor.matmul(sqps[0:1, :], ones_bf[:], g2[:],
                                 start=(mz == 0), stop=(mz == NKE - 1))
            rsrow = stat2.tile([1, LSEQ], F32, tag="rsrow")
            nc.scalar.activation(out=rsrow[:], in_=sqps[0:1, :],
                                 func=AF.Ln, scale=1.0 / E,
                                 bias=nepsc[0:1, :])
            nc.scalar.activation(out=rsrow[:], in_=rsrow[:],
                                 func=AF.Exp, scale=-0.5)
            nc.sync.dma_start(out=rs_d[:], in_=rsrow[:])
            rsbc = pGa.tile([128, LSEQ], F32, tag="rsbc")
            nc.sync.dma_start(out=rsbc[:], in_=row_bcast(rs_d[0:1, :]))
            for mz in range(NKE):
                eng = nc.gpsimd if mz % 4 == 3 else nc.vector
                eng.tensor_mul(szT[:, mz, :], yg[:, mz, :], rsbc[:])
            for tt in range(NCHUNK):
                for half in range(2):
                    ps = psO.tile([128, 512], F32, tag="po")
                    for k in range(NKE):
                        nc.tensor.matmul(
                            ps[:], szT[:, k, tt * 128:(tt + 1) * 128],
                            wo_sb[:, k, half * 512:(half + 1) * 512],
                            start=(k == 0), stop=(k == NKE - 1))
                    hsr = pGa.tile([128, 512], F32, tag="hsr")
                    nc.sync.dma_start(
                        out=hsr[:],
                        in_=hs_in[HALO + tt * 128:HALO + (tt + 1) * 128,
                                  half * 512:(half + 1) * 512])
                    nc.vector.tensor_add(
                        h2[:, tt, half * 512:(half + 1) * 512],
                        ps[:], hsr[:])
            # rms2 + transpose (batched activations)
            sq2s = []
            for tt in range(NCHUNK):
                sq2 = pGa.tile([128, H], F32, tag="sq2", name=f"sq2_{tt}",
                               bufs=2)
                ss2 = stat2.tile([128, 1], F32, tag="ss2", name=f"ss2_{tt}")
                nc.scalar.activation(out=sq2[:], in_=h2[:, tt, :],
                                     func=AF.Square, accum_out=ss2[:])
                sq2s.append(ss2)
            rs2s = []
            for tt in range(NCHUNK):
                rs2 = stat2.tile([128, 1], F32, tag="rs2", name=f"rs2_{tt}")
                nc.scalar.activation(out=rs2[:], in_=sq2s[tt][:],
                                     func=AF.Ln, scale=1.0 / H,
                                     bias=epsc[:])
                rs2s.append(rs2)
            for tt in range(NCHUNK):
                nc.scalar.activation(out=rs2s[tt][:], in_=rs2s[tt][:],
                                     func=AF.Exp, scale=-0.5)
            for tt in range(NCHUNK):
                h2n = pGa.tile([128, H], BF16, tag="h2n", bufs=2)
                nc.vector.tensor_scalar_mul(h2n[:], h2[:, tt, :], rs2s[tt][:])
                nc.sync.dma_start_transpose(
                    h2nT[:, :, tt * 128:(tt + 1) * 128], h2n[:])

        es_yt.close()
        es_P1.close()
        pGU = stack.enter_context(tc.tile_pool(name="pGU", bufs=1))
        gu = pGU.tile([128, NFT, LSEQ], BF16)

        # ---------------- Phase H: MLP ----------------
        with tc.tile_pool(name="wmP", bufs=3) as wmP, \
             tc.tile_pool(name="psM", bufs=4, space="PSUM") as psM, \
             tc.tile_pool(name="pM", bufs=3) as pM:
            for mf in range(NFT):
                wg_m = wmP.tile([128, NKH, 128], BF16, tag="wg")
                nc.sync.dma_start(out=wg_m[:],
                                  in_=wgT[mf * 128:(mf + 1) * 128, :])
                wu_m = wmP.tile([128, NKH, 128], BF16, tag="wu")
                nc.sync.dma_start(out=wu_m[:],
                                  in_=wuT[mf * 128:(mf + 1) * 128, :])
                gps = psM.tile([128, LSEQ], F32, tag="gps")
                for k in range(NKH):
                    nc.tensor.matmul(gps[:], wg_m[:, k, :], h2nT[:, k, :],
                                     start=(k == 0), stop=(k == NKH - 1))
                sg = pM.tile([128, LSEQ], BF16, tag="sg")
                nc.scalar.activation(out=sg[:], in_=gps[:], func=AF.Silu)
                ups = psM.tile([128, LSEQ], F32, tag="ups")
                for k in range(NKH):
                    nc.tensor.matmul(ups[:], wu_m[:, k, :], h2nT[:, k, :],
                                     start=(k == 0), stop=(k == NKH - 1))
                nc.vector.tensor_mul(gu[:, mf, :], sg[:], ups[:])
        with tc.tile_pool(name="wdP", bufs=3) as wdP, \
             tc.tile_pool(name="psD", bufs=1, space="PSUM") as psD, \
             tc.tile_pool(name="pO", bufs=4) as pO:
            dps = []
            for i in range(8):
                dpt = psD.tile([128, 512], F32, tag=f"dp{i}", name=f"dp{i}")
                dps.append(dpt)
            for k in range(NFT):
                wd_k = wdP.tile([128, H], BF16, tag="wd")
                nc.sync.dma_start(out=wd_k[:],
                                  in_=wdT[k * 128:(k + 1) * 128, :])
                for tt in range(NCHUNK):
                    for half in range(2):
                        nc.tensor.matmul(
                            dps[tt * 2 + half][:],
                            gu[:, k, tt * 128:(tt + 1) * 128],
                            wd_k[:, half * 512:(half + 1) * 512],
                            start=(k == 0), stop=(k == NFT - 1))
            for tt in range(NCHUNK):
                for half in range(2):
                    ob = pO.tile([128, 512], F32, tag="ob")
                    nc.vector.tensor_add(
                        ob[:], dps[tt * 2 + half][:],
                        h2[:, tt, half * 512:(half + 1) * 512])
                    nc.sync.dma_start(
                        out=out_d[tt * 128:(tt + 1) * 128,
                                  half * 512:(half + 1) * 512],
                        in_=ob[:])

    nc.finalize()
    return nc


_CACHE = {}


def _get_program():
    if "p" not in _CACHE:
        _CACHE["p"] = build_program(None)
    return _CACHE["p"]


def kernel(hidden_states, w_ln1, w_in, w_conv, b_conv, dt_bias, A_log, D,
           w_mnorm, w_out, w_ln2, w_gate, w_up, w_down):
    bf = ml_dtypes.bfloat16
    hs = np.asarray(hidden_states, np.float32)
    wiTn = (np.asarray(w_in, np.float32) *
            np.asarray(w_ln1, np.float32)[None, :]).T.astype(bf)  # [H, D_IN]
    # reorder columns: xBC (E..E+CONV), dt (E+CONV..), z (0..E)
    perm = np.concatenate([np.arange(E, E + CONV),
                           np.arange(E + CONV, D_IN),
                           np.arange(0, E)])
    wi_perm = wiTn[:, perm]
    # pack into groups of <=512 cols, each group padded to 512
    wi_pad = np.zeros((H, NGRP * 512), bf)
    src = 0
    for gi, (g0, g1, cols) in enumerate(GROUPS):
        wi_pad[:, gi * 512:gi * 512 + cols] = wi_perm[:, src:src + cols]
        src += cols
    wiTn = wi_pad.reshape(NKH, 128, NGRP, 512).transpose(2, 1, 0, 3) \
        .reshape(NGRP * 128, NKH * 512)
    woTn = (np.asarray(w_out, np.float32) *
            np.asarray(w_mnorm, np.float32)[None, :]).T.astype(bf)
    wgTn = (np.asarray(w_gate, np.float32) *
            np.asarray(w_ln2, np.float32)[None, :]).T.astype(bf)
    wuTn = (np.asarray(w_up, np.float32) *
            np.asarray(w_ln2, np.float32)[None, :]).T.astype(bf)
    wgTn = wgTn.reshape(NKH, 128, NFT, 128).transpose(2, 1, 0, 3) \
        .reshape(NFT * 128, NKH * 128)
    wuTn = wuTn.reshape(NKH, 128, NFT, 128).transpose(2, 1, 0, 3) \
        .reshape(NFT * 128, NKH * 128)
    wdTn = np.asarray(w_down, np.float32).T.astype(bf)
    wconv = np.asarray(w_conv, np.float32).reshape(NXT, 128, KC) \
        .transpose(1, 0, 2).reshape(128, NXT * KC).copy()
    bconv = np.asarray(b_conv, np.float32).reshape(NXT, 128).T.copy()
    avec = (-np.exp(np.asarray(A_log, np.float32))).reshape(NH, 1)
    dtb = np.asarray(dt_bias, np.float32).reshape(NH, 1)
    # negu[s, q] = 0 if q >= s else NEG  (kills above-diagonal in eL)
    negu = np.where(np.arange(128)[None, :] >= np.arange(128)[:, None],
                    0.0, NEG).astype(np.float32)
    # dexpc[p, j] = D[head of channel j*128+p]
    ch = (np.arange(NZT)[None, :] * 128 + np.arange(128)[:, None]) // P
    dexpc = np.asarray(D, np.float32)[ch].astype(np.float32).copy()
    idf = np.eye(128, dtype=np.float32)

    nc = _get_program()

    shared = dict(wiT=np.ascontiguousarray(wiTn),
                  woT=np.ascontiguousarray(woTn),
                  wgT=np.ascontiguousarray(wgTn),
                  wuT=np.ascontiguousarray(wuTn),
                  wdT=np.ascontiguousarray(wdTn),
                  wconv=wconv, bconv=bconv, avec=avec, dtb=dtb,
                  negu=negu, idf32=idf, dexpc=dexpc)
    in_maps = []
    for core in range(NCORES):
        b, r = core // 4, core % 4
        s0 = r * LSEQ
        hpad = np.zeros((NROW * 128, H), np.float32)
        hpad[HALO:HALO + LSEQ] = hs[b, s0:s0 + LSEQ]
        if s0 > 0:
            hpad[0:HALO] = hs[b, s0 - HALO:s0]
        m8 = np.zeros((128, 8), np.float32)
        for j in range(4):
            m8[:, j] = 1.0 if j < r else 0.0
            m8[:, 4 + j] = 0.0 if j < r else 1.0
        in_maps.append(dict(shared, hs=hpad, mask8=m8))

    res = run_bass_kernel_spmd(nc, in_maps, list(range(NCORES)))
    out = np.empty((2, 2048, H), np.float32)
    for core in range(NCORES):
        b, r = core // 4, core % 4
        out[b, r * LSEQ:(r + 1) * LSEQ] = res.results[core]["out"]
    return out


# revision 24
# speedup vs baseline: 1.0853x; 1.0853x over previous
"""Trainium2 Bass kernel for NemotronFlash Mamba decoder layer.

Sharding: 8 cores = 2 batches x 4 sequence shards of 512 tokens.
All compute is shard-local except the SSD inter-chunk state, which is
exchanged via one AllGather of (L_k, D_k) within each 4-core batch group.

v2 restructure vs baseline:
- in-proj computes xBC tiles first, dt, then z; conv/dt/states/collective
  start early and overlap the z matmuls.
- activations batched by function (fewer ACT table loads); Rsqrt/Softplus.
- Y produced directly in [E, tokens] layout via PSUM accumulation of
  Y_diag+Y_off per head (no yT DMA transposes, no separate add pass).
- exp(acs) broadcast via a single tiny exp + bf16 DRAM broadcast load.
- gpsimd (Pool engine) offloads part of conv and elementwise work.
"""
import sys
import numpy as np

sys.path.insert(0, "/opt/trn_rl_repo")

from contextlib import ExitStack  # noqa: E402
import ml_dtypes  # noqa: E402
import concourse.bass as bass  # noqa: E402
import concourse.mybir as mybir  # noqa: E402
import concourse.tile as tile  # noqa: E402
from concourse import bacc  # noqa: E402
from concourse.bass_utils import run_bass_kernel_spmd  # noqa: E402

F32 = mybir.dt.float32
BF16 = mybir.dt.bfloat16
AF = mybir.ActivationFunctionType
OP = mybir.AluOpType

H = 1024
E = 2048
NH = 32
P = 64
NST = 128          # d_state
KC = 4             # d_conv
Q = 128            # chunk len
FF = 4096
CONV = E + 2 * NST          # 2304
D_IN = 2 * E + 2 * NST + NH  # 4384
EPS = 1e-6
NEPS = 1e-5
LSEQ = 512         # tokens per shard
NCHUNK = LSEQ // Q  # 4
NROW = 5           # 5 row tiles of 128 = 640 padded rows
HALO = 3
NCORES = 8
NEG = -1.0e30

NZT = E // Q       # 16 z tiles
NXT = CONV // Q    # 18 xBC tiles
NMT = NXT + 1 + NZT  # 35 in-proj tiles (18 xBC + 1 dt + 16 z)
NKH = H // Q       # 8 k tiles over H
NKE = E // Q       # 16 k tiles over E
NFT = FF // Q      # 32 FF tiles
HG = 8             # heads per group
NG = NH // HG      # 4 groups

# in-proj tile order: xBC tiles 0..17, dt (32 rows), z tiles 0..15
TSIZES = [128] * NXT + [32] + [128] * NZT


def make_groups():
    groups = []
    m = 0
    while m < NMT:
        g0 = m
        cols = 0
        while m < NMT and cols + TSIZES[m] <= 512:
            cols += TSIZES[m]
            m += 1
        groups.append((g0, m, cols))
    return groups


GROUPS = make_groups()
NGRP = len(GROUPS)  # 9


def row_bcast(ap_row, parts=128):
    """AP broadcasting a [1, n] row across `parts` partitions (step-0)."""
    return bass.AP(tensor=ap_row.tensor, offset=ap_row.offset,
                   ap=[[0, parts]] + [list(x) for x in ap_row.ap[1:]])


def colbc(src_ap, n, rep):
    # [128, n, rep] broadcast of per-head columns along a new axis
    return bass.AP(tensor=src_ap.tensor, offset=src_ap.offset,
                   ap=[list(src_ap.ap[0])] + [[1, n], [0, rep]])


def rowbc(src_ap, rep, n):
    # [128, rep, n] broadcast of a [128, n] tile along middle axis
    return bass.AP(tensor=src_ap.tensor, offset=src_ap.offset,
                   ap=[list(src_ap.ap[0])] + [[0, rep], [1, n]])


def build_program(dvals):
    nc = bacc.Bacc("TRN2", target_bir_lowering=False, debug=False,
                   num_devices=NCORES)

    hs_in = nc.dram_tensor("hs", [NROW * 128, H], F32, kind="ExternalInput")
    wiT = nc.dram_tensor("wiT", [NGRP * 128, NKH * 512], BF16,
                         kind="ExternalInput")
    woT = nc.dram_tensor("woT", [E, H], BF16, kind="ExternalInput")
    wgT = nc.dram_tensor("wgT", [NFT * 128, NKH * 128], BF16,
                         kind="ExternalInput")
    wuT = nc.dram_tensor("wuT", [NFT * 128, NKH * 128], BF16,
                         kind="ExternalInput")
    wdT = nc.dram_tensor("wdT", [FF, H], BF16, kind="ExternalInput")
    wconv = nc.dram_tensor("wconv", [128, NXT * KC], F32, kind="ExternalInput")
    bconv = nc.dram_tensor("bconv", [128, NXT], F32, kind="ExternalInput")
    avec = nc.dram_tensor("avec", [NH, 1], F32, kind="ExternalInput")
    dtb = nc.dram_tensor("dtb", [NH, 1], F32, kind="ExternalInput")
    mask8 = nc.dram_tensor("mask8", [128, 8], F32, kind="ExternalInput")
    negu = nc.dram_tensor("negu", [128, 128], F32, kind="ExternalInput")
    idf32 = nc.dram_tensor("idf32", [128, 128], F32, kind="ExternalInput")
    dexpc_in = nc.dram_tensor("dexpc", [128, NZT], F32, kind="ExternalInput")
    out_d = nc.dram_tensor("out", [LSEQ, H], F32, kind="ExternalOutput")

    with tile.TileContext(nc) as tc, ExitStack() as stack:
        consts = stack.enter_context(tc.tile_pool(name="consts", bufs=1))
        wconv_sb = consts.tile([128, NXT * KC], F32)
        nc.sync.dma_start(out=wconv_sb[:], in_=wconv[:])
        bconv_sb = consts.tile([128, NXT], F32)
        nc.sync.dma_start(out=bconv_sb[:], in_=bconv[:])
        avec_sb = consts.tile([NH, 1], F32)
        nc.sync.dma_start(out=avec_sb[:], in_=avec[:])
        dtb_sb = consts.tile([NH, 1], F32)
        nc.sync.dma_start(out=dtb_sb[:], in_=dtb[:])
        mask_sb = consts.tile([128, 8], F32)
        nc.sync.dma_start(out=mask_sb[:], in_=mask8[:])
        negu_sb = consts.tile([128, 128], F32)
        nc.sync.dma_start(out=negu_sb[:], in_=negu[:])
        negu_bf = consts.tile([128, 128], BF16)
        nc.vector.tensor_copy(negu_bf[:], negu_sb[:])
        id_sb = consts.tile([128, 128], F32)
        nc.sync.dma_start(out=id_sb[:], in_=idf32[:])
        dexpc_sb = consts.tile([128, NZT], F32)
        nc.sync.dma_start(out=dexpc_sb[:], in_=dexpc_in[:])
        ones_bf = consts.tile([128, 1], BF16)
        nc.vector.memset(ones_bf[:], 1.0)
        zero32 = consts.tile([NH, Q], F32)
        nc.vector.memset(zero32[:], 0.0)
        epsc = consts.tile([128, 1], F32)
        nc.vector.memset(epsc[:], EPS)
        nepsc = consts.tile([128, 1], F32)
        nc.vector.memset(nepsc[:], NEPS)

        ccdram = stack.enter_context(
            tc.tile_pool(name="ccdram", bufs=1, space="DRAM"))
        cc_in = ccdram.tile([128, E + 1], BF16)
        cc_out = ccdram.tile([4, 128, E + 1], BF16)
        acsR_d = ccdram.tile([NCHUNK * NH, Q], F32)
        eacsR_d = ccdram.tile([NCHUNK * NH, Q], BF16)
        rs_d = ccdram.tile([1, LSEQ], F32)
        acst_d = ccdram.tile([1, NCHUNK * 3 * NH], F32)
        drow_d = ccdram.tile([1, 4 * NH], F32)

        big = stack.enter_context(tc.tile_pool(name="big", bufs=1))
        h2 = big.tile([128, NCHUNK, H], F32)
        h2nT = big.tile([128, NKH, LSEQ], BF16)

        es_P1 = ExitStack()                        # A .. end of G
        pP1 = es_P1.enter_context(tc.tile_pool(name="pP1", bufs=1))
        hT = pP1.tile([128, NKH, NROW * 128], BF16)      # h^T  [H, 640]
        szT = pP1.tile([128, NZT, LSEQ], BF16)           # silu(z)^T
        dtraw = pP1.tile([NH, LSEQ], F32)

        # ---------------- Phase A: rmsnorm1 + h^T (batched) ----------------
        with tc.tile_pool(name="pA", bufs=5) as pA, \
             tc.tile_pool(name="stat", bufs=5) as stat:
            hsts, rss = [], []
            for r in range(NROW):
                hst = pA.tile([128, H], F32, tag="hst", name=f"hst{r}")
                nc.sync.dma_start(out=hst[:],
                                  in_=hs_in[r * 128:(r + 1) * 128, :])
                hsts.append(hst)
            ssums = []
            for r in range(NROW):
                sq = pA.tile([128, H], F32, tag="sq", bufs=2, name=f"sq{r}")
                ssum = stat.tile([128, 1], F32, tag="ssum", name=f"ssum{r}")
                nc.scalar.activation(out=sq[:], in_=hsts[r][:],
                                     func=AF.Square, accum_out=ssum[:])
                ssums.append(ssum)
            for r in range(NROW):
                rs = stat.tile([128, 1], F32, tag="rs", name=f"rs{r}")
                nc.scalar.activation(out=rs[:], in_=ssums[r][:],
                                     func=AF.Ln, scale=1.0 / H,
                                     bias=epsc[:])
                rss.append(rs)
            for r in range(NROW):
                nc.scalar.activation(out=rss[r][:], in_=rss[r][:],
                                     func=AF.Exp, scale=-0.5)
            for r in range(NROW):
                hbf = pA.tile([128, H], BF16, tag="hbf", name=f"hbf{r}",
                              bufs=3)
                nc.vector.tensor_scalar_mul(hbf[:], hsts[r][:], rss[r][:])
                nc.sync.dma_start_transpose(
                    hT[:, :, r * 128:(r + 1) * 128], hbf[:])

        es_yt = ExitStack()                        # B .. end of G
        pYT = es_yt.enter_context(tc.tile_pool(name="pYT", bufs=1))
        yT = pYT.tile([128, NKE, LSEQ], BF16)

        es_cf = ExitStack()                        # B .. F
        pCF = es_cf.enter_context(tc.tile_pool(name="pCF", bufs=1))
        dtacsT = pCF.tile([128, NCHUNK, 3 * NH], F32)
        cstates = pCF.tile([128, NCHUNK, E], BF16)
        alast = pCF.tile([128, NCHUNK, NH], F32)
        wdtb = pCF.tile([128, NCHUNK, NH], BF16)
        dcstb = pCF.tile([128, NCHUNK, NH], BF16)
        dt_sb = pCF.tile([NH, LSEQ], F32)
        acs = pCF.tile([NH, LSEQ], F32)

        es_df = ExitStack()                        # B .. F
        pDF = es_df.enter_context(tc.tile_pool(name="pDF", bufs=1))
        xcbc = pDF.tile([128, 2, LSEQ], BF16)
        G_sb = pDF.tile([128, NCHUNK, Q], BF16)
        x_tm = pDF.tile([128, NCHUNK, E], BF16)
        B_tm = pDF.tile([128, NCHUNK, NST], BF16)

        es_dx = ExitStack()                        # B .. yT prefill (xc)
        pDX = es_dx.enter_context(tc.tile_pool(name="pDX", bufs=1))
        xc = pDX.tile([128, NZT, LSEQ], BF16)

        # ---------------- Phase B + C + D + E interleaved ----------------
        es_wip = ExitStack()
        wip = es_wip.enter_context(tc.tile_pool(name="wip", bufs=2))
        psB = ExitStack()
        psBp = psB.enter_context(tc.tile_pool(name="psB", bufs=4,
                                              space="PSUM"))
        psBh = psB.enter_context(tc.tile_pool(name="psBh", bufs=2,
                                              space="PSUM"))
        es_xbc = ExitStack()
        xbcf = es_xbc.enter_context(tc.tile_pool(name="xbcf", bufs=1))
        xbc = xbcf.tile([128, NXT, HALO + LSEQ], BF16)

        def conv_tile(j):
            eng = nc.vector
            acc = None
            for k in range(KC):
                if k == 0:
                    acc = xbcf.tile([128, LSEQ], F32, tag="cacc", bufs=3,
                                    name=f"cacc{j}_0")
                    eng.tensor_scalar_mul(
                        acc[:], xbc[:, j, 0:LSEQ],
                        wconv_sb[:, j * KC:j * KC + 1])
                else:
                    acc2 = xbcf.tile([128, LSEQ], F32, tag="cacc", bufs=3,
                                     name=f"cacc{j}_{k}")
                    eng.scalar_tensor_tensor(
                        out=acc2[:], in0=xbc[:, j, k:k + LSEQ],
                        scalar=wconv_sb[:, j * KC + k:j * KC + k + 1],
                        in1=acc[:], op0=OP.mult, op1=OP.add)
                    acc = acc2
            xdst = (xc[:, j, :] if j < NZT else xcbc[:, j - NZT, :])
            nc.scalar.activation(out=xdst, in_=acc[:], func=AF.Silu,
                                 bias=bconv_sb[:, j:j + 1])
            if j < NZT:
                nc.sync.dma_start_transpose(
                    x_tm[:, :, j * 128:(j + 1) * 128], xc[:, j, :])
            elif j == NZT:
                nc.sync.dma_start_transpose(B_tm[:], xcbc[:, 0, :])

        def emit_group(gi):
            g0, g1, cols = GROUPS[gi]
            wi_g = wip.tile([128, NKH, 512], BF16, tag="wi", name=f"wi{gi}")
            nc.sync.dma_start(out=wi_g[:],
                              in_=wiT[gi * 128:(gi + 1) * 128, :])
            moff = 0
            for mm in range(g0, g1):
                mrows = TSIZES[mm]
                ps = psBp.tile([128, LSEQ], F32, tag="ps")
                for k in range(NKH):
                    nc.tensor.matmul(
                        ps[:mrows, :],
                        wi_g[:, k, moff:moff + mrows],
                        hT[:, k, HALO:HALO + LSEQ],
                        start=(k == 0), stop=(k == NKH - 1))
                if mm < NXT:                      # xBC tile
                    j = mm
                    nc.scalar.copy(xbc[:, j, HALO:], ps[:])
                    psh = psBh.tile([128, HALO], F32, tag="psh")
                    for k in range(NKH):
                        nc.tensor.matmul(
                            psh[:], wi_g[:, k, moff:moff + 128],
                            hT[:, k, 0:HALO],
                            start=(k == 0), stop=(k == NKH - 1))
                    nc.scalar.copy(xbc[:, j, 0:HALO], psh[:])
                    conv_tile(j)
                elif mm == NXT:                   # dt tile
                    nc.vector.tensor_copy(dtraw[:], ps[:NH, :])
                else:                             # z tile: silu from PSUM
                    mz = mm - NXT - 1
                    nc.scalar.activation(out=szT[:, mz, :], in_=ps[:],
                                         func=AF.Silu)
                moff += mrows

        # Part 1: xBC + dt + z0 (groups 0..4), conv pipelined per tile
        for gi in range(5):
            emit_group(gi)
        es_xbc.close()

        # ---------------- Phase C: dt path ----------------
        with tc.tile_pool(name="pC", bufs=2) as pC:
            e1 = pC.tile([NH, LSEQ], F32, tag="e1")
            nc.scalar.activation(out=e1[:], in_=dtraw[:], func=AF.Exp,
                                 bias=dtb_sb[:])
            nc.vector.tensor_scalar_add(e1[:], e1[:], 1.0)
            nc.scalar.activation(out=dt_sb[:], in_=e1[:], func=AF.Ln)
            lndt = pC.tile([NH, LSEQ], F32, tag="lndt")
            nc.scalar.activation(out=lndt[:], in_=dt_sb[:], func=AF.Ln)
            dA = pC.tile([NH, LSEQ], F32, tag="dA")
            nc.vector.tensor_scalar_mul(dA[:], dt_sb[:], avec_sb[:])
            for c in range(NCHUNK):
                nc.vector.tensor_tensor_scan(
                    acs[:, c * Q:(c + 1) * Q], dA[:, c * Q:(c + 1) * Q],
                    zero32[:], 0.0, OP.add, OP.add)
            av = acsR_d[:]
            nc.sync.dma_start(
                out=bass.AP(tensor=av.tensor, offset=av.offset,
                            ap=[[Q, NH], [NH * Q, NCHUNK], [1, Q]]),
                in_=acs[:].rearrange("h (c q) -> h c q", c=NCHUNK))
            stk = pC.tile([3 * NH, LSEQ], F32, tag="stk")
            nc.vector.tensor_copy(stk[0:NH, :], dt_sb[:])
            nc.vector.tensor_copy(stk[NH:2 * NH, :], acs[:])
            nc.vector.tensor_copy(stk[2 * NH:3 * NH, :], lndt[:])
            with tc.tile_pool(name="psC", bufs=2, space="PSUM") as psC:
                for c in range(NCHUNK):
                    pst = psC.tile([128, 3 * NH], F32, tag="pst")
                    nc.tensor.transpose(pst[:], stk[:, c * Q:(c + 1) * Q],
                                        id_sb[0:3 * NH, 0:3 * NH])
                    nc.scalar.copy(dtacsT[:, c, :], pst[:])
            nc.sync.dma_start(out=acst_d[:], in_=dtacsT[127:128, :, :])
            at_ = acst_d[:]
            nc.sync.dma_start(
                out=alast[:],
                in_=bass.AP(tensor=at_.tensor, offset=at_.offset + NH,
                            ap=[[0, 128], [3 * NH, NCHUNK], [1, NH]]))
            dec0 = pC.tile([128, NCHUNK, NH], F32, tag="dec0")
            nc.vector.scalar_tensor_tensor(
                out=dec0[:], in0=dtacsT[:, :, NH:2 * NH], scalar=-1.0,
                in1=alast[:], op0=OP.mult, op1=OP.add)
            # batched Exp set: decT, dcstb, eacs, dkcol
            decT = pC.tile([128, NCHUNK, NH], F32, tag="decT")
            nc.scalar.activation(out=decT[:], in_=dec0[:], func=AF.Exp)
            nc.scalar.activation(out=dcstb[:], in_=alast[:], func=AF.Exp)
            eacs = pC.tile([NH, LSEQ], BF16, tag="eacs")
            nc.scalar.activation(out=eacs[:], in_=acs[:], func=AF.Exp)
            acs4 = acs[:].rearrange("p (c q) -> p c q", c=NCHUNK)[:, :, Q - 1]
            asum = pC.tile([NH, 1], F32, tag="asum")
            nc.vector.tensor_reduce(asum[:], acs4, axis=mybir.AxisListType.X,
                                    op=OP.add)
            dkcol = pC.tile([NH, 1], BF16, tag="dkcol")
            nc.scalar.activation(out=dkcol[:], in_=asum[:], func=AF.Exp)
            eav = eacsR_d[:]
            nc.sync.dma_start(
                out=bass.AP(tensor=eav.tensor, offset=eav.offset,
                            ap=[[Q, NH], [NH * Q, NCHUNK], [1, Q]]),
                in_=eacs[:].rearrange("h (c q) -> h c q", c=NCHUNK))
            nc.vector.tensor_mul(wdtb[:], decT[:], dtacsT[:, :, 0:NH])
            dcol = pCF.tile([128, NCHUNK, NH], F32, name="dcol")
            nc.vector.tensor_sub(dcol[:], dtacsT[:, :, NH:2 * NH],
                                 dtacsT[:, :, 2 * NH:3 * NH])

            # G_sb = B^T C per chunk (unmasked; eL handles causality)
            with tc.tile_pool(name="psGm", bufs=2, space="PSUM") as psGm:
                for c in range(NCHUNK):
                    gps = psGm.tile([128, Q], F32, tag="gps")
                    nc.tensor.matmul(gps[:], xcbc[:, 0, c * Q:(c + 1) * Q],
                                     xcbc[:, 1, c * Q:(c + 1) * Q],
                                     start=True, stop=True)
                    nc.vector.tensor_copy(G_sb[:, c, :], gps[:])

            emit_group(5)

            # ------------- Phase E: states + collective -------------
            with tc.tile_pool(name="psE", bufs=2, space="PSUM") as psE, \
                 tc.tile_pool(name="pE", bufs=2) as pE:
                xv = [x_tm[:, c, :].rearrange("p (h q) -> p h q", h=NH)
                      for c in range(NCHUNK)]
                for c in range(NCHUNK):
                    xdd = pE.tile([128, NH, P], BF16, tag="xdd",
                                  name=f"xdd{c}", bufs=1)
                    nc.vector.tensor_mul(xdd[:], xv[c],
                                           colbc(wdtb[:, c, :], NH, P))
                    for g in range(NG):
                        ps_st = psE.tile([128, 512], F32, tag="ps_st")
                        nc.tensor.matmul(
                            ps_st[:], B_tm[:, c, :],
                            xdd[:, g * HG:(g + 1) * HG, :],
                            start=True, stop=True)
                        if g % 2 == 0:
                            nc.scalar.copy(
                                cstates[:, c, g * 512:(g + 1) * 512],
                                ps_st[:])
                        else:
                            nc.vector.tensor_copy(
                                cstates[:, c, g * 512:(g + 1) * 512],
                                ps_st[:])
                # L combine via suffix decay products
                dsuf = pE.tile([128, NCHUNK, NH], F32, tag="dsuf", bufs=1)
                nc.vector.tensor_copy(dsuf[:, 3, :], dcstb[:, 3, :])
                nc.vector.tensor_mul(dsuf[:, 2, :], dsuf[:, 3, :],
                                     dcstb[:, 2, :])
                nc.vector.tensor_mul(dsuf[:, 1, :], dsuf[:, 2, :],
                                     dcstb[:, 1, :])
                cs_v = [cstates[:, c, :].rearrange("p (h q) -> p h q", h=NH)
                        for c in range(NCHUNK)]
                m0 = pE.tile([128, NH, P], BF16, tag="lwork", bufs=3,
                             name="m0")
                nc.vector.tensor_mul(m0[:], cs_v[0],
                                     colbc(dsuf[:, 1, :], NH, P))
                m1 = pE.tile([128, NH, P], BF16, tag="lwork", bufs=3,
                             name="m1")
                nc.vector.tensor_mul(m1[:], cs_v[1],
                                     colbc(dsuf[:, 2, :], NH, P))
                m2 = pE.tile([128, NH, P], BF16, tag="lwork", bufs=3,
                             name="m2")
                nc.vector.tensor_mul(m2[:], cs_v[2],
                                     colbc(dsuf[:, 3, :], NH, P))
                nc.vector.tensor_add(m0[:], m0[:], m1[:])
                nc.vector.tensor_add(m2[:], m2[:], cs_v[3])
                Lbf = pE.tile([128, E], BF16, tag="lbf", bufs=1)
                nc.vector.tensor_add(
                    Lbf[:].rearrange("p (h q) -> p h q", h=NH),
                    m0[:], m2[:])
                nc.gpsimd.dma_start(out=cc_in[:, 0:E], in_=Lbf[:])
                nc.gpsimd.dma_start(out=cc_in[0:NH, E:E + 1], in_=dkcol[:])
                nc.gpsimd.collective_compute(
                    "AllGather", OP.bypass,
                    replica_groups=[[0, 1, 2, 3], [4, 5, 6, 7]],
                    ins=[cc_in.opt()], outs=[cc_out.opt()])

        # Part 2: remaining z tiles (groups 6..8)
        for gi in range(6, NGRP):
            emit_group(gi)
        psB.close()
        es_wip.close()

        # yT prefill with D-skip term: yT[:, j, :] = D * x
        for j in range(NZT):
            nc.vector.tensor_scalar_mul(yT[:, j, :], xc[:, j, :],
                                        dexpc_sb[:, j:j + 1])
        es_dx.close()

        es_mt = ExitStack()                        # Fprep .. F
        pMT = es_mt.enter_context(tc.tile_pool(name="pMT", bufs=1))
        mt_sb = pMT.tile([128, NCHUNK, NH, Q], BF16)

        # ------- Phase F-prep: mt (overlaps collective; vector/scalar only,
        # gpsimd queue is blocked inside collective_compute) -------
        with tc.tile_pool(name="pFp", bufs=2) as pFp:
            for c in range(NCHUNK):
                for g in range(NG):
                    h0 = g * HG
                    R_all = pFp.tile([128, HG, Q], F32, tag="rall")
                    av2 = acsR_d[:]
                    nc.sync.dma_start(
                        out=R_all[:],
                        in_=bass.AP(tensor=av2.tensor,
                                    offset=av2.offset + (c * NH + h0) * Q,
                                    ap=[[0, 128], [1, HG * Q]]))
                    # seg = acs_q - acs_s  (f32 in, bf16 out)
                    # seg = acs_q - acs_s + ln(dt_s)  (dt folded in exp)
                    seg = pFp.tile([128, HG, Q], BF16, tag="seg")
                    nc.vector.scalar_tensor_tensor(
                        out=seg[:],
                        in0=colbc(dcol[:, c, h0:h0 + HG], HG, Q),
                        scalar=-1.0, in1=R_all[:],
                        op0=OP.mult, op1=OP.add)
                    segm = pFp.tile([128, HG, Q], BF16, tag="segm")
                    nc.vector.tensor_add(segm[:], seg[:],
                                         rowbc(negu_bf[:], HG, Q))
                    eL = pFp.tile([128, HG, Q], BF16, tag="eL")
                    nc.scalar.activation(out=eL[:], in_=segm[:], func=AF.Exp)
                    nc.vector.tensor_mul(
                        mt_sb[:, c, h0:h0 + HG, :], eL[:],
                        rowbc(G_sb[:, c, :], HG, Q))

        # ---------------- S_init combine ----------------
        es_sb = ExitStack()
        sbfp = es_sb.enter_context(tc.tile_pool(name="sbfp", bufs=2))
        Sbf = None
        with tc.tile_pool(name="pS", bufs=1) as pS:
            Lg = pS.tile([128, 4, E], BF16, tag="Lg")
            Dg = pS.tile([NH, 4], BF16, tag="Dg")
            for j in range(4):
                nc.sync.dma_start(out=Lg[:, j, :], in_=cc_out[j, :, 0:E])
                nc.sync.dma_start(out=Dg[:, j:j + 1],
                                  in_=cc_out[j, 0:NH, E:E + 1])
            deff = pS.tile([NH, 4], F32, tag="deff")
            for j in range(4):
                nc.vector.scalar_tensor_tensor(
                    out=deff[:, j:j + 1], in0=Dg[:, j:j + 1],
                    scalar=mask_sb[0:NH, j:j + 1],
                    in1=mask_sb[0:NH, 4 + j:5 + j],
                    op0=OP.mult, op1=OP.add)
            # coef[j] = mask_j * prod_{i>j} deff_i (suffix products, tiny)
            suf = pS.tile([NH, 4], F32, tag="suf")
            nc.vector.memset(suf[:, 3:4], 1.0)
            nc.vector.tensor_copy(suf[:, 2:3], deff[:, 3:4])
            nc.vector.tensor_mul(suf[:, 1:2], deff[:, 2:3], suf[:, 2:3])
            nc.vector.tensor_mul(suf[:, 0:1], deff[:, 1:2], suf[:, 1:2])
            coef = pS.tile([NH, 4], F32, tag="coef")
            nc.vector.tensor_mul(coef[:], suf[:], mask_sb[0:NH, 0:4])
            for j in range(4):
                nc.sync.dma_start(out=drow_d[0:1, j * NH:(j + 1) * NH],
                                  in_=coef[:, j:j + 1])
            dbc = pS.tile([128, 4 * NH], F32, tag="dbc")
            nc.sync.dma_start(out=dbc[:], in_=row_bcast(drow_d[0:1, :]))
            lgv = [Lg[:, j, :].rearrange("p (h q) -> p h q", h=NH)
                   for j in range(4)]
            ps0 = pS.tile([128, NH, P], BF16, tag="sw0")
            nc.vector.tensor_mul(ps0[:], lgv[0], colbc(dbc[:, 0:NH], NH, P))
            ps1 = pS.tile([128, NH, P], BF16, tag="sw1")
            nc.vector.tensor_mul(ps1[:], lgv[1],
                                 colbc(dbc[:, NH:2 * NH], NH, P))
            ps2 = pS.tile([128, NH, P], BF16, tag="sw2")
            nc.vector.tensor_mul(ps2[:], lgv[2],
                                 colbc(dbc[:, 2 * NH:3 * NH], NH, P))
            ps3 = pS.tile([128, NH, P], BF16, tag="sw3")
            nc.vector.tensor_mul(ps3[:], lgv[3],
                                 colbc(dbc[:, 3 * NH:4 * NH], NH, P))
            a01 = pS.tile([128, NH, P], BF16, tag="sa01")
            nc.vector.tensor_add(a01[:], ps0[:], ps1[:])
            a23 = pS.tile([128, NH, P], BF16, tag="sa23")
            nc.vector.tensor_add(a23[:], ps2[:], ps3[:])
            Sbf = sbfp.tile([128, E], BF16, tag="sbf", name="sbf0")
            nc.vector.tensor_add(
                Sbf[:].rearrange("p (h q) -> p h q", h=NH), a01[:], a23[:])

        # ---------------- Phase F: merged Y_diag+Y_off -> yT ----------
        with tc.tile_pool(name="pF2", bufs=2) as pF2, \
             tc.tile_pool(name="psY", bufs=8, space="PSUM") as psY:
            for c in range(NCHUNK):
                # Ct = exp(acs) * C  (independent of collective)
                Ct = pF2.tile([128, NH, Q], BF16, tag="ct", bufs=1)
                for g in range(NG):
                    h0 = g * HG
                    eA = pF2.tile([128, HG, Q], BF16, tag="ear")
                    eav2 = eacsR_d[:]
                    nc.sync.dma_start(
                        out=eA[:],
                        in_=bass.AP(tensor=eav2.tensor,
                                    offset=eav2.offset + (c * NH + h0) * Q,
                                    ap=[[0, 128], [1, HG * Q]]))
                    nc.vector.tensor_mul(
                        Ct[:, h0:h0 + HG, :], eA[:],
                        rowbc(xcbc[:, 1, c * Q:(c + 1) * Q], HG, Q))
                for t in range(4):
                    yps = psY.tile([128, 512], F32, tag="yps")
                    for jj in range(4):
                        j = 4 * t + jj
                        for hh in range(2):
                            h = 2 * j + hh
                            reg = yps[hh * 64:(hh + 1) * 64,
                                      jj * 128:(jj + 1) * 128]
                            nc.tensor.matmul(
                                reg, x_tm[:, c, h * P:(h + 1) * P],
                                mt_sb[:, c, h, :],
                                start=True, stop=False)
                            nc.tensor.matmul(
                                reg, Sbf[:, h * P:(h + 1) * P],
                                Ct[:, h, :],
                                start=False, stop=True)
                    for jj in range(4):
                        j = 4 * t + jj
                        nc.vector.tensor_add(
                            yT[:, j, c * Q:(c + 1) * Q],
                            yT[:, j, c * Q:(c + 1) * Q],
                            yps[:, jj * 128:(jj + 1) * 128])
                # next state (chain) - after this chunk's Y_off matmuls
                if c < NCHUNK - 1:
                    Snext = sbfp.tile([128, E], BF16, tag="sbf",
                                      name=f"sbf{c + 1}")
                    st = pF2.tile([128, NH, P], BF16, tag="stmp", bufs=1)
                    nc.vector.tensor_mul(
                        st[:], Sbf[:].rearrange("p (h q) -> p h q", h=NH),
                        colbc(dcstb[:, c, :], NH, P))
                    nc.vector.tensor_add(
                        Snext[:].rearrange("p (h q) -> p h q", h=NH), st[:],
                        cstates[:, c, :].rearrange("p (h q) -> p h q", h=NH))
                    Sbf = Snext
        es_sb.close()
        es_mt.close()
        es_df.close()
        es_cf.close()

        # ---------------- Phase G: gating + norm + out-proj ----------------
        with tc.tile_pool(name="pGa", bufs=2) as pGa, \
             tc.tile_pool(name="ygP", bufs=1) as ygP, \
             tc.tile_pool(name="woP", bufs=1) as woP, \
             tc.tile_pool(name="psN", bufs=1, space="PSUM") as psN, \
             tc.tile_pool(name="psO", bufs=3, space="PSUM") as psO, \
             tc.tile_pool(name="stat2", bufs=4) as stat2:
            wo_sb = woP.tile([128, NKE, H], BF16)
            for k in range(NKE):
                nc.sync.dma_start(out=wo_sb[:, k, :],
                                  in_=woT[k * 128:(k + 1) * 128, :])
            yg = ygP.tile([128, NKE, LSEQ], BF16)
            for mz in range(NKE):
                eng = nc.gpsimd if mz % 4 == 3 else nc.vector
                eng.tensor_mul(yg[:, mz, :], yT[:, mz, :], szT[:, mz, :])
            sqps = psN.tile([128, LSEQ], F32)
            for mz in range(NKE):
                g2 = pGa.tile([128, LSEQ], BF16, tag="g2", bufs=3)
                nc.scalar.activation(out=g2[:], in_=yg[:, mz, :],
                                     func=AF.Square)
                nc.tensor.matmul(sqps[0:1, :], ones_bf[:], g2[:],
                                 start=(mz == 0), stop=(mz == NKE - 1))
            rsrow = stat2.tile([1, LSEQ], F32, tag="rsrow")
            nc.scalar.activation(out=rsrow[:], in_=sqps[0:1, :],
                                 func=AF.Ln, scale=1.0 / E,
                                 bias=nepsc[0:1, :])
            nc.scalar.activation(out=rsrow[:], in_=rsrow[:],
                                 func=AF.Exp, scale=-0.5)
            nc.sync.dma_start(out=rs_d[:], in_=rsrow[:])
            rsbc = pGa.tile([128, LSEQ], F32, tag="rsbc")
            nc.sync.dma_start(out=rsbc[:], in_=row_bcast(rs_d[0:1, :]))
            for mz in range(NKE):
                eng = nc.gpsimd if mz % 4 == 3 else nc.vector
                eng.tensor_mul(szT[:, mz, :], yg[:, mz, :], rsbc[:])
            for tt in range(NCHUNK):
                for half in range(2):
                    ps = psO.tile([128, 512], F32, tag="po")
                    for k in range(NKE):
                        nc.tensor.matmul(
                            ps[:], szT[:, k, tt * 128:(tt + 1) * 128],
                            wo_sb[:, k, half * 512:(half + 1) * 512],
                            start=(k == 0), stop=(k == NKE - 1))
                    hsr = pGa.tile([128, 512], F32, tag="hsr")
                    nc.sync.dma_start(
                        out=hsr[:],
                        in_=hs_in[HALO + tt * 128:HALO + (tt + 1) * 128,
                                  half * 512:(half + 1) * 512])
                    nc.vector.tensor_add(
                        h2[:, tt, half * 512:(half + 1) * 512],
                        ps[:], hsr[:])
            # rms2 + transpose (batched activations)
            sq2s = []
            for tt in range(NCHUNK):
                sq2 = pGa.tile([128, H], F32, tag="sq2", name=f"sq2_{tt}",
                               bufs=2)
                ss2 = stat2.tile([128, 1], F32, tag="ss2", name=f"ss2_{tt}")
                nc.scalar.activation(out=sq2[:], in_=h2[:, tt, :],
                                     func=AF.Square, accum_out=ss2[:])
                sq2s.append(ss2)
            rs2s = []
            for tt in range(NCHUNK):
                rs2 = stat2.tile([128, 1], F32, tag="rs2", name=f"rs2_{tt}")
                nc.scalar.activation(out=rs2[:], in_=sq2s[tt][:],
                                     func=AF.Ln, scale=1.0 / H,
                                     bias=epsc[:])
                rs2s.append(rs2)
            for tt in range(NCHUNK):
                nc.scalar.activation(out=rs2s[tt][:], in_=rs2s[tt][:],
                                     func=AF.Exp, scale=-0.5)
            for tt in range(NCHUNK):
                h2n = pGa.tile([128, H], BF16, tag="h2n", bufs=2)
                nc.vector.tensor_scalar_mul(h2n[:], h2[:, tt, :], rs2s[tt][:])
                nc.sync.dma_start_transpose(
                    h2nT[:, :, tt * 128:(tt + 1) * 128], h2n[:])

        es_yt.close()
        es_P1.close()
        pGU = stack.enter_context(tc.tile_pool(name="pGU", bufs=1))
        gu = pGU.tile([128, NFT, LSEQ], BF16)

        # ---------------- Phase H: MLP ----------------
        with tc.tile_pool(name="wmP", bufs=3) as wmP, \
             tc.tile_pool(name="psM", bufs=4, space="PSUM") as psM, \
             tc.tile_pool(name="pM", bufs=3) as pM:
            for mf in range(NFT):
                wg_m = wmP.tile([128, NKH, 128], BF16, tag="wg")
                nc.sync.dma_start(out=wg_m[:],
                                  in_=wgT[mf * 128:(mf + 1) * 128, :])
                wu_m = wmP.tile([128, NKH, 128], BF16, tag="wu")
                nc.sync.dma_start(out=wu_m[:],
                                  in_=wuT[mf * 128:(mf + 1) * 128, :])
                gps = psM.tile([128, LSEQ], F32, tag="gps")
                for k in range(NKH):
                    nc.tensor.matmul(gps[:], wg_m[:, k, :], h2nT[:, k, :],
                                     start=(k == 0), stop=(k == NKH - 1))
                sg = pM.tile([128, LSEQ], BF16, tag="sg")
                nc.scalar.activation(out=sg[:], in_=gps[:], func=AF.Silu)
                ups = psM.tile([128, LSEQ], F32, tag="ups")
                for k in range(NKH):
                    nc.tensor.matmul(ups[:], wu_m[:, k, :], h2nT[:, k, :],
                                     start=(k == 0), stop=(k == NKH - 1))
                nc.vector.tensor_mul(gu[:, mf, :], sg[:], ups[:])
        with tc.tile_pool(name="wdP", bufs=3) as wdP, \
             tc.tile_pool(name="psD", bufs=1, space="PSUM") as psD, \
             tc.tile_pool(name="pO", bufs=4) as pO:
            dps = []
            for i in range(8):
                dpt = psD.tile([128, 512], F32, tag=f"dp{i}", name=f"dp{i}")
                dps.append(dpt)
            for k in range(NFT):
                wd_k = wdP.tile([128, H], BF16, tag="wd")
                nc.sync.dma_start(out=wd_k[:],
                                  in_=wdT[k * 128:(k + 1) * 128, :])
                for tt in range(NCHUNK):
                    for half in range(2):
                        nc.tensor.matmul(
                            dps[tt * 2 + half][:],
                            gu[:, k, tt * 128:(tt + 1) * 128],
                            wd_k[:, half * 512:(half + 1) * 512],
                            start=(k == 0), stop=(k == NFT - 1))
            for tt in range(NCHUNK):
                for half in range(2):
                    ob = pO.tile([128, 512], F32, tag="ob")
                    nc.vector.tensor_add(
                        ob[:], dps[tt * 2 + half][:],
                        h2[:, tt, half * 512:(half + 1) * 512])
                    nc.sync.dma_start(
                        out=out_d[tt * 128:(tt + 1) * 128,
                                  half * 512:(half + 1) * 512],
                        in_=ob[:])

    nc.finalize()
    return nc


_CACHE = {}


def _get_program():
    if "p" not in _CACHE:
        _CACHE["p"] = build_program(None)
    return _CACHE["p"]


def kernel(hidden_states, w_ln1, w_in, w_conv, b_conv, dt_bias, A_log, D,
           w_mnorm, w_out, w_ln2, w_gate, w_up, w_down):
    bf = ml_dtypes.bfloat16
    hs = np.asarray(hidden_states, np.float32)
    wiTn = (np.asarray(w_in, np.float32) *
            np.asarray(w_ln1, np.float32)[None, :]).T.astype(bf)  # [H, D_IN]
    # reorder columns: xBC (E..E+CONV), dt (E+CONV..), z (0..E)
    perm = np.concatenate([np.arange(E, E + CONV),
                           np.arange(E + CONV, D_IN),
                           np.arange(0, E)])
    wi_perm = wiTn[:, perm]
    # pack into groups of <=512 cols, each group padded to 512
    wi_pad = np.zeros((H, NGRP * 512), bf)
    src = 0
    for gi, (g0, g1, cols) in enumerate(GROUPS):
        wi_pad[:, gi * 512:gi * 512 + cols] = wi_perm[:, src:src + cols]
        src += cols
    wiTn = wi_pad.reshape(NKH, 128, NGRP, 512).transpose(2, 1, 0, 3) \
        .reshape(NGRP * 128, NKH * 512)
    woTn = (np.asarray(w_out, np.float32) *
            np.asarray(w_mnorm, np.float32)[None, :]).T.astype(bf)
    wgTn = (np.asarray(w_gate, np.float32) *
            np.asarray(w_ln2, np.float32)[None, :]).T.astype(bf)
    wuTn = (np.asarray(w_up, np.float32) *
            np.asarray(w_ln2, np.float32)[None, :]).T.astype(bf)
    wgTn = wgTn.reshape(NKH, 128, NFT, 128).transpose(2, 1, 0, 3) \
        .reshape(NFT * 128, NKH * 128)
    wuTn = wuTn.reshape(NKH, 128, NFT, 128).transpose(2, 1, 0, 3) \
        .reshape(NFT * 128, NKH * 128)
    wdTn = np.asarray(w_down, np.float32).T.astype(bf)
    wconv = np.asarray(w_conv, np.float32).reshape(NXT, 128, KC) \
        .transpose(1, 0, 2).reshape(128, NXT * KC).copy()
    bconv = np.asarray(b_conv, np.float32).reshape(NXT, 128).T.copy()
    avec = (-np.exp(np.asarray(A_log, np.float32))).reshape(NH, 1)
    dtb = np.asarray(dt_bias, np.float32).reshape(NH, 1)
    # negu[s, q] = 0 if q >= s else NEG  (kills above-diagonal in eL)
    negu = np.where(np.arange(128)[None, :] >= np.arange(128)[:, None],
                    0.0, NEG).astype(np.float32)
    # dexpc[p, j] = D[head of channel j*128+p]
    ch = (np.arange(NZT)[None, :] * 128 + np.arange(128)[:, None]) // P
    dexpc = np.asarray(D, np.float32)[ch].astype(np.float32).copy()
    idf = np.eye(128, dtype=np.float32)

    nc = _get_program()

    shared = dict(wiT=np.ascontiguousarray(wiTn),
                  woT=np.ascontiguousarray(woTn),
                  wgT=np.ascontiguousarray(wgTn),
                  wuT=np.ascontiguousarray(wuTn),
                  wdT=np.ascontiguousarray(wdTn),
                  wconv=wconv, bconv=bconv, avec=avec, dtb=dtb,
                  negu=negu, idf32=idf, dexpc=dexpc)
    in_maps = []
    for core in range(NCORES):
        b, r = core // 4, core % 4
        s0 = r * LSEQ
        hpad = np.zeros((NROW * 128, H), np.float32)
        hpad[HALO:HALO + LSEQ] = hs[b, s0:s0 + LSEQ]
        if s0 > 0:
            hpad[0:HALO] = hs[b, s0 - HALO:s0]
        m8 = np.zeros((128, 8), np.float32)
        for j in range(4):
            m8[:, j] = 1.0 if j < r else 0.0
            m8[:, 4 + j] = 0.0 if j < r else 1.0
        in_maps.append(dict(shared, hs=hpad, mask8=m8))

    res = run_bass_kernel_spmd(nc, in_maps, list(range(NCORES)))
    out = np.empty((2, 2048, H), np.float32)
    for core in range(NCORES):
        b, r = core // 4, core % 4
        out[b, r * LSEQ:(r + 1) * LSEQ] = res.results[core]["out"]
    return out


# revision 26
# speedup vs baseline: 1.2669x; 1.1673x over previous
"""Trainium2 Bass kernel for NemotronFlash Mamba decoder layer.

Sharding: 8 cores = 2 batches x 4 sequence shards of 512 tokens.
All compute is shard-local except the SSD inter-chunk state, which is
exchanged via one AllGather of (L_k, D_k) within each 4-core batch group.

v2 restructure vs baseline:
- in-proj computes xBC tiles first, dt, then z; conv/dt/states/collective
  start early and overlap the z matmuls.
- activations batched by function (fewer ACT table loads); Rsqrt/Softplus.
- Y produced directly in [E, tokens] layout via PSUM accumulation of
  Y_diag+Y_off per head (no yT DMA transposes, no separate add pass).
- exp(acs) broadcast via a single tiny exp + bf16 DRAM broadcast load.
- gpsimd (Pool engine) offloads part of conv and elementwise work.
"""
import sys
import numpy as np

sys.path.insert(0, "/opt/trn_rl_repo")

from contextlib import ExitStack  # noqa: E402
import ml_dtypes  # noqa: E402
import concourse.bass as bass  # noqa: E402
import concourse.mybir as mybir  # noqa: E402
import concourse.tile as tile  # noqa: E402
from concourse import bacc  # noqa: E402
from concourse.bass_utils import run_bass_kernel_spmd  # noqa: E402

F32 = mybir.dt.float32
BF16 = mybir.dt.bfloat16
FP8 = mybir.dt.float8e4
PM = mybir.MatmulPerfMode
AF = mybir.ActivationFunctionType
OP = mybir.AluOpType

H = 1024
E = 2048
NH = 32
P = 64
NST = 128          # d_state
KC = 4             # d_conv
Q = 128            # chunk len
FF = 4096
CONV = E + 2 * NST          # 2304
D_IN = 2 * E + 2 * NST + NH  # 4384
EPS = 1e-6
NEPS = 1e-5
LSEQ = 512         # tokens per shard
NCHUNK = LSEQ // Q  # 4
NROW = 5           # 5 row tiles of 128 = 640 padded rows
HALO = 3
NCORES = 8
NEG = -1.0e30

NZT = E // Q       # 16 z tiles
NXT = CONV // Q    # 18 xBC tiles
NMT = NXT + 1 + NZT  # 35 in-proj tiles (18 xBC + 1 dt + 16 z)
NKH = H // Q       # 8 k tiles over H
NKE = E // Q       # 16 k tiles over E
NFT = FF // Q      # 32 FF tiles
HG = 8             # heads per group
NG = NH // HG      # 4 groups

# in-proj tile order: xBC tiles 0..17, dt (32 rows), z tiles 0..15
TSIZES = [128] * NXT + [32] + [128] * NZT


def make_groups():
    groups = []
    m = 0
    while m < NMT:
        g0 = m
        cols = 0
        while m < NMT and cols + TSIZES[m] <= 512:
            cols += TSIZES[m]
            m += 1
        groups.append((g0, m, cols))
    return groups


GROUPS = make_groups()
NGRP = len(GROUPS)  # 9


def row_bcast(ap_row, parts=128):
    """AP broadcasting a [1, n] row across `parts` partitions (step-0)."""
    return bass.AP(tensor=ap_row.tensor, offset=ap_row.offset,
                   ap=[[0, parts]] + [list(x) for x in ap_row.ap[1:]])


def colbc(src_ap, n, rep):
    # [128, n, rep] broadcast of per-head columns along a new axis
    return bass.AP(tensor=src_ap.tensor, offset=src_ap.offset,
                   ap=[list(src_ap.ap[0])] + [[1, n], [0, rep]])


def rowbc(src_ap, rep, n):
    # [128, rep, n] broadcast of a [128, n] tile along middle axis
    return bass.AP(tensor=src_ap.tensor, offset=src_ap.offset,
                   ap=[list(src_ap.ap[0])] + [[0, rep], [1, n]])


def build_program(dvals):
    nc = bacc.Bacc("TRN2", target_bir_lowering=False, debug=False,
                   num_devices=NCORES)

    hs_in = nc.dram_tensor("hs", [NROW * 128, H], F32, kind="ExternalInput")
    wiT = nc.dram_tensor("wiT", [NGRP * 128, NKH * 512], BF16,
                         kind="ExternalInput")
    woT = nc.dram_tensor("woT", [E, H], BF16, kind="ExternalInput")
    wgT = nc.dram_tensor("wgT", [NFT * 128, NKH * 128], FP8,
                         kind="ExternalInput")
    wuT = nc.dram_tensor("wuT", [NFT * 128, NKH * 128], FP8,
                         kind="ExternalInput")
    wdT = nc.dram_tensor("wdT", [FF, H], FP8, kind="ExternalInput")
    wconv = nc.dram_tensor("wconv", [128, NXT * KC], F32, kind="ExternalInput")
    bconv = nc.dram_tensor("bconv", [128, NXT], F32, kind="ExternalInput")
    avec = nc.dram_tensor("avec", [NH, 1], F32, kind="ExternalInput")
    dtb = nc.dram_tensor("dtb", [NH, 1], F32, kind="ExternalInput")
    mask8 = nc.dram_tensor("mask8", [128, 8], F32, kind="ExternalInput")
    negu = nc.dram_tensor("negu", [128, 128], F32, kind="ExternalInput")
    idf32 = nc.dram_tensor("idf32", [128, 128], F32, kind="ExternalInput")
    dexpc_in = nc.dram_tensor("dexpc", [128, NZT], F32, kind="ExternalInput")
    out_d = nc.dram_tensor("out", [LSEQ, H], F32, kind="ExternalOutput")

    with tile.TileContext(nc) as tc, ExitStack() as stack:
        consts = stack.enter_context(tc.tile_pool(name="consts", bufs=1))
        wconv_sb = consts.tile([128, NXT * KC], F32)
        nc.sync.dma_start(out=wconv_sb[:], in_=wconv[:])
        bconv_sb = consts.tile([128, NXT], F32)
        nc.sync.dma_start(out=bconv_sb[:], in_=bconv[:])
        avec_sb = consts.tile([NH, 1], F32)
        nc.sync.dma_start(out=avec_sb[:], in_=avec[:])
        dtb_sb = consts.tile([NH, 1], F32)
        nc.sync.dma_start(out=dtb_sb[:], in_=dtb[:])
        mask_sb = consts.tile([128, 8], F32)
        nc.sync.dma_start(out=mask_sb[:], in_=mask8[:])
        negu_sb = consts.tile([128, 128], F32)
        nc.sync.dma_start(out=negu_sb[:], in_=negu[:])
        negu_bf = consts.tile([128, 128], BF16)
        nc.vector.tensor_copy(negu_bf[:], negu_sb[:])
        id_sb = consts.tile([128, 128], F32)
        nc.sync.dma_start(out=id_sb[:], in_=idf32[:])
        dexpc_sb = consts.tile([128, NZT], F32)
        nc.sync.dma_start(out=dexpc_sb[:], in_=dexpc_in[:])
        ones_bf = consts.tile([128, 1], BF16)
        nc.vector.memset(ones_bf[:], 1.0)
        zero32 = consts.tile([NH, Q], F32)
        nc.vector.memset(zero32[:], 0.0)
        epsc = consts.tile([128, 1], F32)
        nc.vector.memset(epsc[:], EPS)
        nepsc = consts.tile([128, 1], F32)
        nc.vector.memset(nepsc[:], NEPS)

        ccdram = stack.enter_context(
            tc.tile_pool(name="ccdram", bufs=1, space="DRAM"))
        cc_in = ccdram.tile([128, E + 1], BF16)
        cc_out = ccdram.tile([4, 128, E + 1], BF16)
        acsR_d = ccdram.tile([NCHUNK * NH, Q], F32)
        eacsR_d = ccdram.tile([NCHUNK * NH, Q], BF16)
        rs_d = ccdram.tile([1, LSEQ], F32)
        acst_d = ccdram.tile([1, NCHUNK * 3 * NH], F32)
        drow_d = ccdram.tile([1, 4 * NH], F32)

        big = stack.enter_context(tc.tile_pool(name="big", bufs=1))
        h2 = big.tile([128, NCHUNK, H], F32)
        h2nT = big.tile([128, NKH, LSEQ], BF16)
        h2nT8 = big.tile([128, NKH, LSEQ], FP8)

        es_P1 = ExitStack()                        # A .. end of G
        pP1 = es_P1.enter_context(tc.tile_pool(name="pP1", bufs=1))
        hT = pP1.tile([128, NKH, NROW * 128], BF16)      # h^T  [H, 640]
        szT = pP1.tile([128, NZT, LSEQ], BF16)           # silu(z)^T
        dtraw = pP1.tile([NH, LSEQ], F32)

        # ---------------- Phase A: rmsnorm1 + h^T (batched) ----------------
        with tc.tile_pool(name="pA", bufs=5) as pA, \
             tc.tile_pool(name="stat", bufs=5) as stat:
            hsts, rss = [], []
            for r in range(NROW):
                hst = pA.tile([128, H], F32, tag="hst", name=f"hst{r}")
                nc.sync.dma_start(out=hst[:],
                                  in_=hs_in[r * 128:(r + 1) * 128, :])
                hsts.append(hst)
            ssums = []
            for r in range(NROW):
                sq = pA.tile([128, H], F32, tag="sq", bufs=2, name=f"sq{r}")
                ssum = stat.tile([128, 1], F32, tag="ssum", name=f"ssum{r}")
                nc.scalar.activation(out=sq[:], in_=hsts[r][:],
                                     func=AF.Square, accum_out=ssum[:])
                ssums.append(ssum)
            for r in range(NROW):
                rs = stat.tile([128, 1], F32, tag="rs", name=f"rs{r}")
                nc.scalar.activation(out=rs[:], in_=ssums[r][:],
                                     func=AF.Ln, scale=1.0 / H,
                                     bias=epsc[:])
                rss.append(rs)
            for r in range(NROW):
                nc.scalar.activation(out=rss[r][:], in_=rss[r][:],
                                     func=AF.Exp, scale=-0.5)
            for r in range(NROW):
                hbf = pA.tile([128, H], BF16, tag="hbf", name=f"hbf{r}",
                              bufs=3)
                nc.vector.tensor_scalar_mul(hbf[:], hsts[r][:], rss[r][:])
                nc.sync.dma_start_transpose(
                    hT[:, :, r * 128:(r + 1) * 128], hbf[:])

        es_yt = ExitStack()                        # B .. end of G
        pYT = es_yt.enter_context(tc.tile_pool(name="pYT", bufs=1))
        yT = pYT.tile([128, NKE, LSEQ], BF16)

        es_cf = ExitStack()                        # B .. F
        pCF = es_cf.enter_context(tc.tile_pool(name="pCF", bufs=1))
        dtacsT = pCF.tile([128, NCHUNK, 3 * NH], F32)
        cstates = pCF.tile([128, NCHUNK, E], BF16)
        alast = pCF.tile([128, NCHUNK, NH], F32)
        wdtb = pCF.tile([128, NCHUNK, NH], BF16)
        dcstb = pCF.tile([128, NCHUNK, NH], BF16)
        dt_sb = pCF.tile([NH, LSEQ], F32)
        acs = pCF.tile([NH, LSEQ], F32)

        es_df = ExitStack()                        # B .. F
        pDF = es_df.enter_context(tc.tile_pool(name="pDF", bufs=1))
        xcbc = pDF.tile([128, 2, LSEQ], BF16)
        G_sb = pDF.tile([128, NCHUNK, Q], BF16)
        x_tm = pDF.tile([128, NCHUNK, E], BF16)
        B_tm = pDF.tile([128, NCHUNK, NST], BF16)

        es_dx = ExitStack()                        # B .. yT prefill (xc)
        pDX = es_dx.enter_context(tc.tile_pool(name="pDX", bufs=1))
        xc = pDX.tile([128, NZT, LSEQ], BF16)

        # ---------------- Phase B + C + D + E interleaved ----------------
        es_wip = ExitStack()
        wip = es_wip.enter_context(tc.tile_pool(name="wip", bufs=2))
        psB = ExitStack()
        psBp = psB.enter_context(tc.tile_pool(name="psB", bufs=4,
                                              space="PSUM"))
        psBh = psB.enter_context(tc.tile_pool(name="psBh", bufs=2,
                                              space="PSUM"))
        es_xbc = ExitStack()
        xbcf = es_xbc.enter_context(tc.tile_pool(name="xbcf", bufs=1))
        xbc = xbcf.tile([128, NXT, HALO + LSEQ], BF16)

        def conv_tile(j):
            eng = nc.vector
            acc = None
            for k in range(KC):
                if k == 0:
                    acc = xbcf.tile([128, LSEQ], F32, tag="cacc", bufs=3,
                                    name=f"cacc{j}_0")
                    eng.tensor_scalar_mul(
                        acc[:], xbc[:, j, 0:LSEQ],
                        wconv_sb[:, j * KC:j * KC + 1])
                else:
                    acc2 = xbcf.tile([128, LSEQ], F32, tag="cacc", bufs=3,
                                     name=f"cacc{j}_{k}")
                    eng.scalar_tensor_tensor(
                        out=acc2[:], in0=xbc[:, j, k:k + LSEQ],
                        scalar=wconv_sb[:, j * KC + k:j * KC + k + 1],
                        in1=acc[:], op0=OP.mult, op1=OP.add)
                    acc = acc2
            xdst = (xc[:, j, :] if j < NZT else xcbc[:, j - NZT, :])
            nc.scalar.activation(out=xdst, in_=acc[:], func=AF.Silu,
                                 bias=bconv_sb[:, j:j + 1])
            if j < NZT:
                nc.sync.dma_start_transpose(
                    x_tm[:, :, j * 128:(j + 1) * 128], xc[:, j, :])
            elif j == NZT:
                nc.sync.dma_start_transpose(B_tm[:], xcbc[:, 0, :])

        def emit_group(gi):
            g0, g1, cols = GROUPS[gi]
            wi_g = wip.tile([128, NKH, 512], BF16, tag="wi", name=f"wi{gi}")
            nc.sync.dma_start(out=wi_g[:],
                              in_=wiT[gi * 128:(gi + 1) * 128, :])
            moff = 0
            for mm in range(g0, g1):
                mrows = TSIZES[mm]
                ps = psBp.tile([128, LSEQ], F32, tag="ps")
                for k in range(NKH):
                    nc.tensor.matmul(
                        ps[:mrows, :],
                        wi_g[:, k, moff:moff + mrows],
                        hT[:, k, HALO:HALO + LSEQ],
                        start=(k == 0), stop=(k == NKH - 1))
                if mm < NXT:                      # xBC tile
                    j = mm
                    nc.scalar.copy(xbc[:, j, HALO:], ps[:])
                    psh = psBh.tile([128, HALO], F32, tag="psh")
                    for k in range(NKH):
                        nc.tensor.matmul(
                            psh[:], wi_g[:, k, moff:moff + 128],
                            hT[:, k, 0:HALO],
                            start=(k == 0), stop=(k == NKH - 1))
                    nc.scalar.copy(xbc[:, j, 0:HALO], psh[:])
                    conv_tile(j)
                elif mm == NXT:                   # dt tile
                    nc.vector.tensor_copy(dtraw[:], ps[:NH, :])
                else:                             # z tile: silu from PSUM
                    mz = mm - NXT - 1
                    nc.scalar.activation(out=szT[:, mz, :], in_=ps[:],
                                         func=AF.Silu)
                moff += mrows

        # Part 1: xBC + dt + z0 (groups 0..4), conv pipelined per tile
        for gi in range(5):
            emit_group(gi)
        es_xbc.close()

        # ---------------- Phase C: dt path ----------------
        with tc.tile_pool(name="pC", bufs=2) as pC:
            e1 = pC.tile([NH, LSEQ], F32, tag="e1")
            nc.scalar.activation(out=e1[:], in_=dtraw[:], func=AF.Exp,
                                 bias=dtb_sb[:])
            nc.vector.tensor_scalar_add(e1[:], e1[:], 1.0)
            nc.scalar.activation(out=dt_sb[:], in_=e1[:], func=AF.Ln)
            lndt = pC.tile([NH, LSEQ], F32, tag="lndt")
            nc.scalar.activation(out=lndt[:], in_=dt_sb[:], func=AF.Ln)
            dA = pC.tile([NH, LSEQ], F32, tag="dA")
            nc.vector.tensor_scalar_mul(dA[:], dt_sb[:], avec_sb[:])
            for c in range(NCHUNK):
                nc.vector.tensor_tensor_scan(
                    acs[:, c * Q:(c + 1) * Q], dA[:, c * Q:(c + 1) * Q],
                    zero32[:], 0.0, OP.add, OP.add)
            av = acsR_d[:]
            nc.sync.dma_start(
                out=bass.AP(tensor=av.tensor, offset=av.offset,
                            ap=[[Q, NH], [NH * Q, NCHUNK], [1, Q]]),
                in_=acs[:].rearrange("h (c q) -> h c q", c=NCHUNK))
            stk = pC.tile([3 * NH, LSEQ], F32, tag="stk")
            nc.vector.tensor_copy(stk[0:NH, :], dt_sb[:])
            nc.vector.tensor_copy(stk[NH:2 * NH, :], acs[:])
            nc.vector.tensor_copy(stk[2 * NH:3 * NH, :], lndt[:])
            with tc.tile_pool(name="psC", bufs=2, space="PSUM") as psC:
                for c in range(NCHUNK):
                    pst = psC.tile([128, 3 * NH], F32, tag="pst")
                    nc.tensor.transpose(pst[:], stk[:, c * Q:(c + 1) * Q],
                                        id_sb[0:3 * NH, 0:3 * NH])
                    nc.scalar.copy(dtacsT[:, c, :], pst[:])
            nc.sync.dma_start(out=acst_d[:], in_=dtacsT[127:128, :, :])
            at_ = acst_d[:]
            nc.sync.dma_start(
                out=alast[:],
                in_=bass.AP(tensor=at_.tensor, offset=at_.offset + NH,
                            ap=[[0, 128], [3 * NH, NCHUNK], [1, NH]]))
            dec0 = pC.tile([128, NCHUNK, NH], F32, tag="dec0")
            nc.vector.scalar_tensor_tensor(
                out=dec0[:], in0=dtacsT[:, :, NH:2 * NH], scalar=-1.0,
                in1=alast[:], op0=OP.mult, op1=OP.add)
            # batched Exp set: decT, dcstb, eacs, dkcol
            decT = pC.tile([128, NCHUNK, NH], F32, tag="decT")
            nc.scalar.activation(out=decT[:], in_=dec0[:], func=AF.Exp)
            nc.scalar.activation(out=dcstb[:], in_=alast[:], func=AF.Exp)
            eacs = pC.tile([NH, LSEQ], BF16, tag="eacs")
            nc.scalar.activation(out=eacs[:], in_=acs[:], func=AF.Exp)
            acs4 = acs[:].rearrange("p (c q) -> p c q", c=NCHUNK)[:, :, Q - 1]
            asum = pC.tile([NH, 1], F32, tag="asum")
            nc.vector.tensor_reduce(asum[:], acs4, axis=mybir.AxisListType.X,
                                    op=OP.add)
            dkcol = pC.tile([NH, 1], BF16, tag="dkcol")
            nc.scalar.activation(out=dkcol[:], in_=asum[:], func=AF.Exp)
            eav = eacsR_d[:]
            nc.sync.dma_start(
                out=bass.AP(tensor=eav.tensor, offset=eav.offset,
                            ap=[[Q, NH], [NH * Q, NCHUNK], [1, Q]]),
                in_=eacs[:].rearrange("h (c q) -> h c q", c=NCHUNK))
            nc.vector.tensor_mul(wdtb[:], decT[:], dtacsT[:, :, 0:NH])
            dcol = pCF.tile([128, NCHUNK, NH], F32, name="dcol")
            nc.vector.tensor_sub(dcol[:], dtacsT[:, :, NH:2 * NH],
                                 dtacsT[:, :, 2 * NH:3 * NH])

            # G_sb = B^T C per chunk (unmasked; eL handles causality)
            with tc.tile_pool(name="psGm", bufs=2, space="PSUM") as psGm:
                for c in range(NCHUNK):
                    gps = psGm.tile([128, Q], F32, tag="gps")
                    nc.tensor.matmul(gps[:], xcbc[:, 0, c * Q:(c + 1) * Q],
                                     xcbc[:, 1, c * Q:(c + 1) * Q],
                                     start=True, stop=True)
                    nc.vector.tensor_copy(G_sb[:, c, :], gps[:])

            emit_group(5)

            # ------------- Phase E: states + collective -------------
            with tc.tile_pool(name="psE", bufs=2, space="PSUM") as psE, \
                 tc.tile_pool(name="pE", bufs=2) as pE:
                xv = [x_tm[:, c, :].rearrange("p (h q) -> p h q", h=NH)
                      for c in range(NCHUNK)]
                for c in range(NCHUNK):
                    xdd = pE.tile([128, NH, P], BF16, tag="xdd",
                                  name=f"xdd{c}", bufs=1)
                    nc.vector.tensor_mul(xdd[:], xv[c],
                                           colbc(wdtb[:, c, :], NH, P))
                    for g in range(NG):
                        ps_st = psE.tile([128, 512], F32, tag="ps_st")
                        nc.tensor.matmul(
                            ps_st[:], B_tm[:, c, :],
                            xdd[:, g * HG:(g + 1) * HG, :],
                            start=True, stop=True)
                        if g % 2 == 0:
                            nc.scalar.copy(
                                cstates[:, c, g * 512:(g + 1) * 512],
                                ps_st[:])
                        else:
                            nc.vector.tensor_copy(
                                cstates[:, c, g * 512:(g + 1) * 512],
                                ps_st[:])
                # L combine via suffix decay products
                dsuf = pE.tile([128, NCHUNK, NH], F32, tag="dsuf", bufs=1)
                nc.vector.tensor_copy(dsuf[:, 3, :], dcstb[:, 3, :])
                nc.vector.tensor_mul(dsuf[:, 2, :], dsuf[:, 3, :],
                                     dcstb[:, 2, :])
                nc.vector.tensor_mul(dsuf[:, 1, :], dsuf[:, 2, :],
                                     dcstb[:, 1, :])
                cs_v = [cstates[:, c, :].rearrange("p (h q) -> p h q", h=NH)
                        for c in range(NCHUNK)]
                m0 = pE.tile([128, NH, P], BF16, tag="lwork", bufs=3,
                             name="m0")
                nc.vector.tensor_mul(m0[:], cs_v[0],
                                     colbc(dsuf[:, 1, :], NH, P))
                m1 = pE.tile([128, NH, P], BF16, tag="lwork", bufs=3,
                             name="m1")
                nc.vector.tensor_mul(m1[:], cs_v[1],
                                     colbc(dsuf[:, 2, :], NH, P))
                m2 = pE.tile([128, NH, P], BF16, tag="lwork", bufs=3,
                             name="m2")
                nc.vector.tensor_mul(m2[:], cs_v[2],
                                     colbc(dsuf[:, 3, :], NH, P))
                nc.vector.tensor_add(m0[:], m0[:], m1[:])
                nc.vector.tensor_add(m2[:], m2[:], cs_v[3])
                Lbf = pE.tile([128, E], BF16, tag="lbf", bufs=1)
                nc.vector.tensor_add(
                    Lbf[:].rearrange("p (h q) -> p h q", h=NH),
                    m0[:], m2[:])
                nc.gpsimd.dma_start(out=cc_in[:, 0:E], in_=Lbf[:])
                nc.gpsimd.dma_start(out=cc_in[0:NH, E:E + 1], in_=dkcol[:])
                nc.gpsimd.collective_compute(
                    "AllGather", OP.bypass,
                    replica_groups=[[0, 1, 2, 3], [4, 5, 6, 7]],
                    ins=[cc_in.opt()], outs=[cc_out.opt()])

        # Part 2: remaining z tiles (groups 6..8)
        for gi in range(6, NGRP):
            emit_group(gi)
        psB.close()
        es_wip.close()

        # yT prefill with D-skip term: yT[:, j, :] = D * x
        for j in range(NZT):
            nc.vector.tensor_scalar_mul(yT[:, j, :], xc[:, j, :],
                                        dexpc_sb[:, j:j + 1])
        es_dx.close()

        es_mt = ExitStack()                        # Fprep .. F
        pMT = es_mt.enter_context(tc.tile_pool(name="pMT", bufs=1))
        mt_sb = pMT.tile([128, NCHUNK, NH, Q], BF16)

        # ------- Phase F-prep: mt (overlaps collective; vector/scalar only,
        # gpsimd queue is blocked inside collective_compute) -------
        with tc.tile_pool(name="pFp", bufs=2) as pFp:
            for c in range(NCHUNK):
                for g in range(NG):
                    h0 = g * HG
                    R_all = pFp.tile([128, HG, Q], F32, tag="rall")
                    av2 = acsR_d[:]
                    nc.sync.dma_start(
                        out=R_all[:],
                        in_=bass.AP(tensor=av2.tensor,
                                    offset=av2.offset + (c * NH + h0) * Q,
                                    ap=[[0, 128], [1, HG * Q]]))
                    # seg = acs_q - acs_s  (f32 in, bf16 out)
                    # seg = acs_q - acs_s + ln(dt_s)  (dt folded in exp)
                    seg = pFp.tile([128, HG, Q], BF16, tag="seg")
                    nc.vector.scalar_tensor_tensor(
                        out=seg[:],
                        in0=colbc(dcol[:, c, h0:h0 + HG], HG, Q),
                        scalar=-1.0, in1=R_all[:],
                        op0=OP.mult, op1=OP.add)
                    segm = pFp.tile([128, HG, Q], BF16, tag="segm")
                    nc.vector.tensor_add(segm[:], seg[:],
                                         rowbc(negu_bf[:], HG, Q))
                    eL = pFp.tile([128, HG, Q], BF16, tag="eL")
                    nc.scalar.activation(out=eL[:], in_=segm[:], func=AF.Exp)
                    nc.vector.tensor_mul(
                        mt_sb[:, c, h0:h0 + HG, :], eL[:],
                        rowbc(G_sb[:, c, :], HG, Q))

        # ---------------- S_init combine ----------------
        es_sb = ExitStack()
        sbfp = es_sb.enter_context(tc.tile_pool(name="sbfp", bufs=2))
        Sbf = None
        with tc.tile_pool(name="pS", bufs=1) as pS:
            Lg = pS.tile([128, 4, E], BF16, tag="Lg")
            Dg = pS.tile([NH, 4], BF16, tag="Dg")
            for j in range(4):
                nc.sync.dma_start(out=Lg[:, j, :], in_=cc_out[j, :, 0:E])
                nc.sync.dma_start(out=Dg[:, j:j + 1],
                                  in_=cc_out[j, 0:NH, E:E + 1])
            deff = pS.tile([NH, 4], F32, tag="deff")
            for j in range(4):
                nc.vector.scalar_tensor_tensor(
                    out=deff[:, j:j + 1], in0=Dg[:, j:j + 1],
                    scalar=mask_sb[0:NH, j:j + 1],
                    in1=mask_sb[0:NH, 4 + j:5 + j],
                    op0=OP.mult, op1=OP.add)
            # coef[j] = mask_j * prod_{i>j} deff_i (suffix products, tiny)
            suf = pS.tile([NH, 4], F32, tag="suf")
            nc.vector.memset(suf[:, 3:4], 1.0)
            nc.vector.tensor_copy(suf[:, 2:3], deff[:, 3:4])
            nc.vector.tensor_mul(suf[:, 1:2], deff[:, 2:3], suf[:, 2:3])
            nc.vector.tensor_mul(suf[:, 0:1], deff[:, 1:2], suf[:, 1:2])
            coef = pS.tile([NH, 4], F32, tag="coef")
            nc.vector.tensor_mul(coef[:], suf[:], mask_sb[0:NH, 0:4])
            for j in range(4):
                nc.sync.dma_start(out=drow_d[0:1, j * NH:(j + 1) * NH],
                                  in_=coef[:, j:j + 1])
            dbc = pS.tile([128, 4 * NH], F32, tag="dbc")
            nc.sync.dma_start(out=dbc[:], in_=row_bcast(drow_d[0:1, :]))
            lgv = [Lg[:, j, :].rearrange("p (h q) -> p h q", h=NH)
                   for j in range(4)]
            ps0 = pS.tile([128, NH, P], BF16, tag="sw0")
            nc.vector.tensor_mul(ps0[:], lgv[0], colbc(dbc[:, 0:NH], NH, P))
            ps1 = pS.tile([128, NH, P], BF16, tag="sw1")
            nc.vector.tensor_mul(ps1[:], lgv[1],
                                 colbc(dbc[:, NH:2 * NH], NH, P))
            ps2 = pS.tile([128, NH, P], BF16, tag="sw2")
            nc.vector.tensor_mul(ps2[:], lgv[2],
                                 colbc(dbc[:, 2 * NH:3 * NH], NH, P))
            ps3 = pS.tile([128, NH, P], BF16, tag="sw3")
            nc.vector.tensor_mul(ps3[:], lgv[3],
                                 colbc(dbc[:, 3 * NH:4 * NH], NH, P))
            a01 = pS.tile([128, NH, P], BF16, tag="sa01")
            nc.vector.tensor_add(a01[:], ps0[:], ps1[:])
            a23 = pS.tile([128, NH, P], BF16, tag="sa23")
            nc.vector.tensor_add(a23[:], ps2[:], ps3[:])
            Sbf = sbfp.tile([128, E], BF16, tag="sbf", name="sbf0")
            nc.vector.tensor_add(
                Sbf[:].rearrange("p (h q) -> p h q", h=NH), a01[:], a23[:])

        # ---------------- Phase F: merged Y_diag+Y_off -> yT ----------
        with tc.tile_pool(name="pF2", bufs=2) as pF2, \
             tc.tile_pool(name="psY", bufs=8, space="PSUM") as psY:
            for c in range(NCHUNK):
                # Ct = exp(acs) * C  (independent of collective)
                Ct = pF2.tile([128, NH, Q], BF16, tag="ct", bufs=1)
                for g in range(NG):
                    h0 = g * HG
                    eA = pF2.tile([128, HG, Q], BF16, tag="ear")
                    eav2 = eacsR_d[:]
                    nc.sync.dma_start(
                        out=eA[:],
                        in_=bass.AP(tensor=eav2.tensor,
                                    offset=eav2.offset + (c * NH + h0) * Q,
                                    ap=[[0, 128], [1, HG * Q]]))
                    nc.vector.tensor_mul(
                        Ct[:, h0:h0 + HG, :], eA[:],
                        rowbc(xcbc[:, 1, c * Q:(c + 1) * Q], HG, Q))
                for t in range(4):
                    yps = psY.tile([128, 512], F32, tag="yps")
                    for jj in range(4):
                        j = 4 * t + jj
                        for hh in range(2):
                            h = 2 * j + hh
                            reg = yps[hh * 64:(hh + 1) * 64,
                                      jj * 128:(jj + 1) * 128]
                            nc.tensor.matmul(
                                reg, x_tm[:, c, h * P:(h + 1) * P],
                                mt_sb[:, c, h, :],
                                start=True, stop=False)
                            nc.tensor.matmul(
                                reg, Sbf[:, h * P:(h + 1) * P],
                                Ct[:, h, :],
                                start=False, stop=True)
                    for jj in range(4):
                        j = 4 * t + jj
                        nc.vector.tensor_add(
                            yT[:, j, c * Q:(c + 1) * Q],
                            yT[:, j, c * Q:(c + 1) * Q],
                            yps[:, jj * 128:(jj + 1) * 128])
                # next state (chain) - after this chunk's Y_off matmuls
                if c < NCHUNK - 1:
                    Snext = sbfp.tile([128, E], BF16, tag="sbf",
                                      name=f"sbf{c + 1}")
                    st = pF2.tile([128, NH, P], BF16, tag="stmp", bufs=1)
                    nc.vector.tensor_mul(
                        st[:], Sbf[:].rearrange("p (h q) -> p h q", h=NH),
                        colbc(dcstb[:, c, :], NH, P))
                    nc.vector.tensor_add(
                        Snext[:].rearrange("p (h q) -> p h q", h=NH), st[:],
                        cstates[:, c, :].rearrange("p (h q) -> p h q", h=NH))
                    Sbf = Snext
        es_sb.close()
        es_mt.close()
        es_df.close()
        es_cf.close()

        # ---------------- Phase G: gating + norm + out-proj ----------------
        with tc.tile_pool(name="pGa", bufs=2) as pGa, \
             tc.tile_pool(name="ygP", bufs=1) as ygP, \
             tc.tile_pool(name="woP", bufs=1) as woP, \
             tc.tile_pool(name="psN", bufs=1, space="PSUM") as psN, \
             tc.tile_pool(name="psO", bufs=3, space="PSUM") as psO, \
             tc.tile_pool(name="stat2", bufs=4) as stat2:
            wo_sb = woP.tile([128, NKE, H], BF16)
            for k in range(NKE):
                nc.sync.dma_start(out=wo_sb[:, k, :],
                                  in_=woT[k * 128:(k + 1) * 128, :])
            yg = ygP.tile([128, NKE, LSEQ], BF16)
            for mz in range(NKE):
                eng = nc.gpsimd if mz % 4 == 3 else nc.vector
                eng.tensor_mul(yg[:, mz, :], yT[:, mz, :], szT[:, mz, :])
            sqps = psN.tile([128, LSEQ], F32)
            for mz in range(NKE):
                g2 = pGa.tile([128, LSEQ], BF16, tag="g2", bufs=3)
                nc.scalar.activation(out=g2[:], in_=yg[:, mz, :],
                                     func=AF.Square)
                nc.tensor.matmul(sqps[0:1, :], ones_bf[:], g2[:],
                                 start=(mz == 0), stop=(mz == NKE - 1))
            rsrow = stat2.tile([1, LSEQ], F32, tag="rsrow")
            nc.scalar.activation(out=rsrow[:], in_=sqps[0:1, :],
                                 func=AF.Ln, scale=1.0 / E,
                                 bias=nepsc[0:1, :])
            nc.scalar.activation(out=rsrow[:], in_=rsrow[:],
                                 func=AF.Exp, scale=-0.5)
            nc.sync.dma_start(out=rs_d[:], in_=rsrow[:])
            rsbc = pGa.tile([128, LSEQ], F32, tag="rsbc")
            nc.sync.dma_start(out=rsbc[:], in_=row_bcast(rs_d[0:1, :]))
            for mz in range(NKE):
                eng = nc.gpsimd if mz % 4 == 3 else nc.vector
                eng.tensor_mul(szT[:, mz, :], yg[:, mz, :], rsbc[:])
            for tt in range(NCHUNK):
                for half in range(2):
                    ps = psO.tile([128, 512], F32, tag="po")
                    for k in range(NKE):
                        nc.tensor.matmul(
                            ps[:], szT[:, k, tt * 128:(tt + 1) * 128],
                            wo_sb[:, k, half * 512:(half + 1) * 512],
                            start=(k == 0), stop=(k == NKE - 1))
                    hsr = pGa.tile([128, 512], F32, tag="hsr")
                    nc.sync.dma_start(
                        out=hsr[:],
                        in_=hs_in[HALO + tt * 128:HALO + (tt + 1) * 128,
                                  half * 512:(half + 1) * 512])
                    nc.vector.tensor_add(
                        h2[:, tt, half * 512:(half + 1) * 512],
                        ps[:], hsr[:])
            # rms2 + transpose (batched activations)
            sq2s = []
            for tt in range(NCHUNK):
                sq2 = pGa.tile([128, H], F32, tag="sq2", name=f"sq2_{tt}",
                               bufs=2)
                ss2 = stat2.tile([128, 1], F32, tag="ss2", name=f"ss2_{tt}")
                nc.scalar.activation(out=sq2[:], in_=h2[:, tt, :],
                                     func=AF.Square, accum_out=ss2[:])
                sq2s.append(ss2)
            rs2s = []
            for tt in range(NCHUNK):
                rs2 = stat2.tile([128, 1], F32, tag="rs2", name=f"rs2_{tt}")
                nc.scalar.activation(out=rs2[:], in_=sq2s[tt][:],
                                     func=AF.Ln, scale=1.0 / H,
                                     bias=epsc[:])
                rs2s.append(rs2)
            for tt in range(NCHUNK):
                nc.scalar.activation(out=rs2s[tt][:], in_=rs2s[tt][:],
                                     func=AF.Exp, scale=-0.5)
            for tt in range(NCHUNK):
                h2n = pGa.tile([128, H], BF16, tag="h2n", bufs=2)
                nc.vector.tensor_scalar_mul(h2n[:], h2[:, tt, :], rs2s[tt][:])
                nc.sync.dma_start_transpose(
                    h2nT[:, :, tt * 128:(tt + 1) * 128], h2n[:])
            nc.scalar.activation(out=h2nT8[:], in_=h2nT[:], func=AF.Copy,
                                 scale=16.0)

        es_yt.close()
        es_P1.close()
        pGU = stack.enter_context(tc.tile_pool(name="pGU", bufs=1))
        gu = pGU.tile([128, NFT, LSEQ], FP8)

        # ---------------- Phase H: MLP (fp8 DoubleRow) ----------------
        # weights pre-scaled x32, activations x16 -> gate/up psum = 512*true
        # gu stored as 32*true fp8; down psum = 1024*true
        with tc.tile_pool(name="wmP", bufs=3) as wmP, \
             tc.tile_pool(name="psM", bufs=4, space="PSUM") as psM, \
             tc.tile_pool(name="pM", bufs=3) as pM:
            for mf in range(NFT):
                wg_m = wmP.tile([128, NKH, 128], FP8, tag="wg")
                nc.sync.dma_start(out=wg_m[:],
                                  in_=wgT[mf * 128:(mf + 1) * 128, :])
                wu_m = wmP.tile([128, NKH, 128], FP8, tag="wu")
                nc.sync.dma_start(out=wu_m[:],
                                  in_=wuT[mf * 128:(mf + 1) * 128, :])
                gps = psM.tile([128, LSEQ], F32, tag="gps")
                for kp in range(NKH // 2):
                    nc.tensor.matmul(gps[:], wg_m[:, 2 * kp:2 * kp + 2, :],
                                     h2nT8[:, 2 * kp:2 * kp + 2, :],
                                     start=(kp == 0),
                                     stop=(kp == NKH // 2 - 1),
                                     perf_mode=PM.DoubleRow)
                sg = pM.tile([128, LSEQ], BF16, tag="sg")
                nc.scalar.activation(out=sg[:], in_=gps[:], func=AF.Silu,
                                     scale=1.0 / 512)
                ups = psM.tile([128, LSEQ], F32, tag="ups")
                for kp in range(NKH // 2):
                    nc.tensor.matmul(ups[:], wu_m[:, 2 * kp:2 * kp + 2, :],
                                     h2nT8[:, 2 * kp:2 * kp + 2, :],
                                     start=(kp == 0),
                                     stop=(kp == NKH // 2 - 1),
                                     perf_mode=PM.DoubleRow)
                nc.vector.scalar_tensor_tensor(
                    out=gu[:, mf, :], in0=ups[:], scalar=1.0 / 16,
                    in1=sg[:], op0=OP.mult, op1=OP.mult)
        with tc.tile_pool(name="wdP", bufs=3) as wdP, \
             tc.tile_pool(name="psD", bufs=1, space="PSUM") as psD, \
             tc.tile_pool(name="pO", bufs=4) as pO:
            dps = []
            for i in range(8):
                dpt = psD.tile([128, 512], F32, tag=f"dp{i}", name=f"dp{i}")
                dps.append(dpt)
            for kp in range(NFT // 2):
                wd_k = wdP.tile([128, 2, H], FP8, tag="wd")
                wsrc = wdT[:]
                nc.sync.dma_start(
                    out=wd_k[:],
                    in_=bass.AP(tensor=wsrc.tensor,
                                offset=wsrc.offset + 2 * kp * 128 * H,
                                ap=[[H, 128], [128 * H, 2], [1, H]]))
                for tt in range(NCHUNK):
                    for half in range(2):
                        nc.tensor.matmul(
                            dps[tt * 2 + half][:],
                            gu[:, 2 * kp:2 * kp + 2,
                               tt * 128:(tt + 1) * 128],
                            wd_k[:, :, half * 512:(half + 1) * 512],
                            start=(kp == 0), stop=(kp == NFT // 2 - 1),
                            perf_mode=PM.DoubleRow)
            for tt in range(NCHUNK):
                for half in range(2):
                    ob = pO.tile([128, 512], F32, tag="ob")
                    nc.vector.scalar_tensor_tensor(
                        out=ob[:], in0=dps[tt * 2 + half][:],
                        scalar=1.0 / 1024,
                        in1=h2[:, tt, half * 512:(half + 1) * 512],
                        op0=OP.mult, op1=OP.add)
                    nc.sync.dma_start(
                        out=out_d[tt * 128:(tt + 1) * 128,
                                  half * 512:(half + 1) * 512],
                        in_=ob[:])

    nc.finalize()
    return nc


_CACHE = {}


def _get_program():
    if "p" not in _CACHE:
        _CACHE["p"] = build_program(None)
    return _CACHE["p"]


def kernel(hidden_states, w_ln1, w_in, w_conv, b_conv, dt_bias, A_log, D,
           w_mnorm, w_out, w_ln2, w_gate, w_up, w_down):
    bf = ml_dtypes.bfloat16
    hs = np.asarray(hidden_states, np.float32)
    wiTn = (np.asarray(w_in, np.float32) *
            np.asarray(w_ln1, np.float32)[None, :]).T.astype(bf)  # [H, D_IN]
    # reorder columns: xBC (E..E+CONV), dt (E+CONV..), z (0..E)
    perm = np.concatenate([np.arange(E, E + CONV),
                           np.arange(E + CONV, D_IN),
                           np.arange(0, E)])
    wi_perm = wiTn[:, perm]
    # pack into groups of <=512 cols, each group padded to 512
    wi_pad = np.zeros((H, NGRP * 512), bf)
    src = 0
    for gi, (g0, g1, cols) in enumerate(GROUPS):
        wi_pad[:, gi * 512:gi * 512 + cols] = wi_perm[:, src:src + cols]
        src += cols
    wiTn = wi_pad.reshape(NKH, 128, NGRP, 512).transpose(2, 1, 0, 3) \
        .reshape(NGRP * 128, NKH * 512)
    woTn = (np.asarray(w_out, np.float32) *
            np.asarray(w_mnorm, np.float32)[None, :]).T.astype(bf)
    f8 = ml_dtypes.float8_e4m3fn
    wgTn = (32.0 * np.asarray(w_gate, np.float32) *
            np.asarray(w_ln2, np.float32)[None, :]).T.astype(f8)
    wuTn = (32.0 * np.asarray(w_up, np.float32) *
            np.asarray(w_ln2, np.float32)[None, :]).T.astype(f8)
    wgTn = wgTn.reshape(NKH, 128, NFT, 128).transpose(2, 1, 0, 3) \
        .reshape(NFT * 128, NKH * 128)
    wuTn = wuTn.reshape(NKH, 128, NFT, 128).transpose(2, 1, 0, 3) \
        .reshape(NFT * 128, NKH * 128)
    wdTn = (32.0 * np.asarray(w_down, np.float32)).T.astype(f8)
    wconv = np.asarray(w_conv, np.float32).reshape(NXT, 128, KC) \
        .transpose(1, 0, 2).reshape(128, NXT * KC).copy()
    bconv = np.asarray(b_conv, np.float32).reshape(NXT, 128).T.copy()
    avec = (-np.exp(np.asarray(A_log, np.float32))).reshape(NH, 1)
    dtb = np.asarray(dt_bias, np.float32).reshape(NH, 1)
    # negu[s, q] = 0 if q >= s else NEG  (kills above-diagonal in eL)
    negu = np.where(np.arange(128)[None, :] >= np.arange(128)[:, None],
                    0.0, NEG).astype(np.float32)
    # dexpc[p, j] = D[head of channel j*128+p]
    ch = (np.arange(NZT)[None, :] * 128 + np.arange(128)[:, None]) // P
    dexpc = np.asarray(D, np.float32)[ch].astype(np.float32).copy()
    idf = np.eye(128, dtype=np.float32)

    nc = _get_program()

    shared = dict(wiT=np.ascontiguousarray(wiTn),
                  woT=np.ascontiguousarray(woTn),
                  wgT=np.ascontiguousarray(wgTn),
                  wuT=np.ascontiguousarray(wuTn),
                  wdT=np.ascontiguousarray(wdTn),
                  wconv=wconv, bconv=bconv, avec=avec, dtb=dtb,
                  negu=negu, idf32=idf, dexpc=dexpc)
    in_maps = []
    for core in range(NCORES):
        b, r = core // 4, core % 4
        s0 = r * LSEQ
        hpad = np.zeros((NROW * 128, H), np.float32)
        hpad[HALO:HALO + LSEQ] = hs[b, s0:s0 + LSEQ]
        if s0 > 0:
            hpad[0:HALO] = hs[b, s0 - HALO:s0]
        m8 = np.zeros((128, 8), np.float32)
        for j in range(4):
            m8[:, j] = 1.0 if j < r else 0.0
            m8[:, 4 + j] = 0.0 if j < r else 1.0
        in_maps.append(dict(shared, hs=hpad, mask8=m8))

    res = run_bass_kernel_spmd(nc, in_maps, list(range(NCORES)))
    out = np.empty((2, 2048, H), np.float32)
    for core in range(NCORES):
        b, r = core // 4, core % 4
        out[b, r * LSEQ:(r + 1) * LSEQ] = res.results[core]["out"]
    return out
